# revision 13
# baseline (speedup 1.0000x reference)
"""EnergyCarrier (2-layer GRU cell + heads) Trainium2 kernel.

Full inputs in, full outputs out. Data-parallel over 8 NeuronCores:
batch dim B=32768 sharded into 8x4096 rows; GRU/head weights replicated.

On-chip layout is feature-major ([feature_chunk=128 partitions, batch free])
so the GRU matmuls contract over partitions; activations are transposed at
the DRAM boundary with PE-mode transposes.

Precision scheme: the outputs include discrete quantities (is_terminated
bool, spawn_counts int, rounded positions) whose boundary flips dominate
the error budget, so bf16/f32r matmuls (~2^-12 operand truncation) are out.
Instead every large matmul runs as a 3-pass fp16 hi/lo split
(hi.hi + hi.lo + lo.hi accumulated in one PSUM bank): fp16 products are
exact in the PE's e10m23 accumulator and the dropped lo.lo term is
~2^-22 relative, giving fp32-grade results (measured ~8e-7 rel on HW) at
3 cycles/row instead of fp32's 4. Tiny M<=3 matmuls stay plain fp32.

Spawn decisions are computed from the spawn-gate pre-activation (thresholds
0, ln2, ln5 in logit space) instead of the sigmoid output: the ACT sigmoid
table has a 40-ULP budget, while the matmul pre-activation is fp32-grade.
Rounding uses the +/-1.5*2^23 magic-number trick (RNE, matches jnp.round's
half-to-even).
"""

import os
import sys
from contextlib import ExitStack

sys.path.insert(0, "/opt/trn_rl_repo")

import numpy as np

import concourse.bass as bass
import concourse.tile as tile
from concourse import bacc, mybir
from concourse.bass_utils import run_bass_kernel_spmd

AF = mybir.ActivationFunctionType
OP = mybir.AluOpType
F32 = mybir.dt.float32
F16 = mybir.dt.float16

B, H, IN = 32768, 1024, 33
CORES = 8
BS = B // CORES          # rows per core
NT = 512                 # batch-tile columns
NBT = BS // NT
KC = H // 128            # feature chunks
MAGIC = 12582912.0       # 1.5 * 2**23 -> RNE round-to-integer via add/sub
# spawn thresholds in pre-activation (logit) space:
# decided: sigmoid32(g) > 0.5 <=> g > 2^-23 (fp32 rounding boundary near 0.5)
T0 = float(np.float32(2.0 ** -23))
T1 = float(np.float32(np.log(2.0)))   # p > 2/3
T2 = float(np.float32(np.log(5.0)))   # p > 5/6

LAST_RES = None  # BassKernelResults of the most recent run (for test harness)
_PROG = None


def _pack_big(w):
    """[O,K] (O,K mult of 128) -> [O/128, 128, K]; [m,p,k*128+j] = w[m*128+j, k*128+p]."""
    o, k = w.shape
    m, kc = o // 128, k // 128
    w4 = np.asarray(w, np.float32).reshape(m, 128, kc, 128)
    return np.ascontiguousarray(w4.transpose(0, 3, 2, 1).reshape(m, 128, k))


def _pack_kx(w):
    """[O<=128, K] -> [128, (K/128)*O]; [p, k*O+j] = w[j, k*128+p]."""
    o, k = w.shape
    kc = k // 128
    w3 = np.asarray(w, np.float32).reshape(o, kc, 128)
    return np.ascontiguousarray(w3.transpose(2, 1, 0).reshape(128, kc * o))


def _hi(a):
    return np.asarray(a, np.float32).astype(np.float16)


def _lo(a):
    a = np.asarray(a, np.float32)
    return (a - a.astype(np.float16).astype(np.float32)).astype(np.float16)


def _build():
    nc = bacc.Bacc("TRN2", target_bir_lowering=False, debug=False,
                   num_devices=CORES)

    def din(name, shape, dt=F32):
        return nc.dram_tensor(name, list(shape), dt, kind="ExternalInput").ap()

    def dout(name, shape):
        return nc.dram_tensor(name, list(shape), F32, kind="ExternalOutput").ap()

    x_d = din("x", (BS, IN))
    h0hi_d = din("h0hi", (BS, H), F16)
    h0lo_d = din("h0lo", (BS, H), F16)
    h1hi_d = din("h1hi", (BS, H), F16)
    h1lo_d = din("h1lo", (BS, H), F16)
    fa_d = din("fa", (1, BS))
    ident_d = din("ident", (128, 128))
    wih0h_d = din("wih0h", (IN, 3 * H), F16)
    wih0l_d = din("wih0l", (IN, 3 * H), F16)
    whh0h_d = din("whh0h", (24, 128, H), F16)
    whh0l_d = din("whh0l", (24, 128, H), F16)
    wih1h_d = din("wih1h", (24, 128, H), F16)
    wih1l_d = din("wih1l", (24, 128, H), F16)
    whh1h_d = din("whh1h", (24, 128, H), F16)
    whh1l_d = din("whh1l", (24, 128, H), F16)
    epw1h_d = din("epw1h", (4, 128, H), F16)
    epw1l_d = din("epw1l", (4, 128, H), F16)
    epw2_d = din("epw2t", (128, 4))
    ppw1h_d = din("ppw1h", (128, KC * 64), F16)
    ppw1l_d = din("ppw1l", (128, KC * 64), F16)
    ppw2_d = din("ppw2t", (64, 3))
    sgw1h_d = din("sgw1h", (128, KC * 64), F16)
    sgw1l_d = din("sgw1l", (128, KC * 64), F16)
    sgw2_d = din("sgw2t", (64, 1))
    sewh_d = din("sewh", (128, KC), F16)
    sewl_d = din("sewl", (128, KC), F16)
    brz0_d = din("brz0", (128, 16))
    bin0_d = din("bin0", (128, 8))
    bhn0_d = din("bhn0", (128, 8))
    brz1_d = din("brz1", (128, 16))
    bin1_d = din("bin1", (128, 8))
    bhn1_d = din("bhn1", (128, 8))
    epb1_d = din("epb1", (128, 4))
    ppb1_d = din("ppb1", (64, 1))
    sgb1_d = din("sgb1", (64, 1))
    epb2_d = din("epb2", (1, 1))
    ppb2_d = din("ppb2", (1, 3))
    sgb2_d = din("sgb2", (1, 1))
    seb_d = din("seb", (1, 1))

    en_d = dout("energy", (1, BS))
    pos_d = dout("pos", (3, BS))
    fr_d = dout("frac", (1, BS))
    ct_d = dout("counts", (1, BS))
    tm_d = dout("term", (1, BS))
    nh0_d = dout("nh0", (BS, H))
    nh1_d = dout("nh1", (BS, H))

    with tile.TileContext(nc) as tc, ExitStack() as ctx:
        const = ctx.enter_context(tc.tile_pool(name="const", bufs=1))
        hload = ctx.enter_context(tc.tile_pool(name="hload", bufs=3))
        wmm = ctx.enter_context(tc.tile_pool(name="wmm", bufs=14))
        actp = ctx.enter_context(tc.tile_pool(name="actp", bufs=1))
        ewp = ctx.enter_context(tc.tile_pool(name="ewp", bufs=9))
        hnewp = ctx.enter_context(tc.tile_pool(name="hnewp", bufs=3))
        rowp = ctx.enter_context(tc.tile_pool(name="rowp", bufs=10))
        h64p = ctx.enter_context(tc.tile_pool(name="h64p", bufs=4))
        otp = ctx.enter_context(tc.tile_pool(name="otp", bufs=6))
        xtp = ctx.enter_context(tc.tile_pool(name="xtp", bufs=1))
        ep1p = ctx.enter_context(tc.tile_pool(name="ep1p", bufs=1))
        psmm = ctx.enter_context(tc.tile_pool(name="psmm", bufs=6, space="PSUM"))
        pstr = ctx.enter_context(tc.tile_pool(name="pstr", bufs=2, space="PSUM"))

        def ctile(ap_dram, shape, name, dt=F32):
            t = const.tile(shape, dt, tag=name, name=name)
            nc.sync.dma_start(t[:], ap_dram[:])
            return t

        ident_sb = ctile(ident_d, [128, 128], "ident_sb")
        wih0h_sb = ctile(wih0h_d, [IN, 3 * H], "wih0h_sb", F16)
        wih0l_sb = ctile(wih0l_d, [IN, 3 * H], "wih0l_sb", F16)
        ppw1h_sb = ctile(ppw1h_d, [128, KC * 64], "ppw1h_sb", F16)
        ppw1l_sb = ctile(ppw1l_d, [128, KC * 64], "ppw1l_sb", F16)
        sgw1h_sb = ctile(sgw1h_d, [128, KC * 64], "sgw1h_sb", F16)
        sgw1l_sb = ctile(sgw1l_d, [128, KC * 64], "sgw1l_sb", F16)
        sewh_sb = ctile(sewh_d, [128, KC], "sewh_sb", F16)
        sewl_sb = ctile(sewl_d, [128, KC], "sewl_sb", F16)
        epw2_sb = ctile(epw2_d, [128, 4], "epw2_sb")
        ppw2_sb = ctile(ppw2_d, [64, 3], "ppw2_sb")
        sgw2_sb = ctile(sgw2_d, [64, 1], "sgw2_sb")
        brz0_sb = ctile(brz0_d, [128, 16], "brz0_sb")
        bin0_sb = ctile(bin0_d, [128, 8], "bin0_sb")
        bhn0_sb = ctile(bhn0_d, [128, 8], "bhn0_sb")
        brz1_sb = ctile(brz1_d, [128, 16], "brz1_sb")
        bin1_sb = ctile(bin1_d, [128, 8], "bin1_sb")
        bhn1_sb = ctile(bhn1_d, [128, 8], "bhn1_sb")
        epb1_sb = ctile(epb1_d, [128, 4], "epb1_sb")
        ppb1_sb = ctile(ppb1_d, [64, 1], "ppb1_sb")
        sgb1_sb = ctile(sgb1_d, [64, 1], "sgb1_sb")
        epb2_sb = ctile(epb2_d, [1, 1], "epb2_sb")
        ppb2_sb = ctile(ppb2_d, [1, 3], "ppb2_sb")
        sgb2_sb = ctile(sgb2_d, [1, 1], "sgb2_sb")
        seb_sb = ctile(seb_d, [1, 1], "seb_sb")

        copy_ctr = [0]

        def pcopy(dst, src):
            # alternate DVE/ACT so neither engine owns all psum drains
            if copy_ctr[0] % 2 == 0:
                nc.vector.tensor_copy(dst, src)
            else:
                nc.scalar.copy(dst, src)
            copy_ctr[0] += 1

        def split_from(dst_hi, dst_lo, src):
            # hi = fp16(src); lo = fp16(src - hi). src may be PSUM (1 psum
            # operand per DVE op). Exact to ~2^-22 regardless of rounding.
            pcopy(dst_hi, src)
            nc.vector.scalar_tensor_tensor(dst_lo, dst_hi, -1.0, src,
                                           OP.mult, OP.add)

        def load_transposed(dst_hi, dst_lo, hi_dram, lo_dram, b0):
            # dst[p, k*NT + j] <- src[b0+j, 128k+p]; host pre-split fp16 hi/lo,
            # loaded feature-major via hardware DMA transpose (2-byte dtype)
            for k in range(KC):
                # Activation HWDGE queue: keeps xbar-mode transpose DMAs off
                # the Sync queue that streams the weights (Tile serializes
                # xbar-mode transitions per queue)
                nc.scalar.dma_start_transpose(
                    dst_hi[:, k * NT:(k + 1) * NT],
                    hi_dram[b0:b0 + NT, k * 128:(k + 1) * 128])
                nc.scalar.dma_start_transpose(
                    dst_lo[:, k * NT:(k + 1) * NT],
                    lo_dram[b0:b0 + NT, k * 128:(k + 1) * 128])

        def mm3(ps, wh, wl, xh, xl, start, stop):
            nc.tensor.matmul(ps, wh, xh, start=start, stop=False)
            nc.tensor.matmul(ps, wh, xl, start=False, stop=False)
            nc.tensor.matmul(ps, wl, xh, start=False, stop=stop)

        for bt in range(NBT):
            b0 = bt * NT

            # ---- inputs: h_prev^T then x^T (fp16 hi/lo pairs) ----
            h0Th = actp.tile([128, KC * NT], F16, tag="h0Th", name="h0Th",
                             bufs=2)
            h0Tl = actp.tile([128, KC * NT], F16, tag="h0Tl", name="h0Tl",
                             bufs=2)
            load_transposed(h0Th, h0Tl, h0hi_d, h0lo_d, b0)
            h1Th = actp.tile([128, KC * NT], F16, tag="h1Th", name="h1Th",
                             bufs=2)
            h1Tl = actp.tile([128, KC * NT], F16, tag="h1Tl", name="h1Tl",
                             bufs=2)
            load_transposed(h1Th, h1Tl, h1hi_d, h1lo_d, b0)

            xTh = xtp.tile([IN, NT], F16, tag="xTh", name="xTh")
            xTl = xtp.tile([IN, NT], F16, tag="xTl", name="xTl")
            for i in range(NT // 128):
                xb = hload.tile([128, IN], F32, tag="xb", name="xb")
                nc.sync.dma_start(xb[:], x_d[b0 + i * 128:b0 + (i + 1) * 128, :])
                psx = pstr.tile([128, 128], F32, tag="tr", name="ps_tr")
                nc.tensor.transpose(psx[0:IN, :], xb[:], ident_sb[:])
                sl = slice(i * 128, (i + 1) * 128)
                split_from(xTh[:, sl], xTl[:, sl], psx[0:IN, :])

            h0nTh = actp.tile([128, KC * NT], F16, tag="h0nTh", name="h0nTh")
            h0nTl = actp.tile([128, KC * NT], F16, tag="h0nTl", name="h0nTl")
            h1nTh = actp.tile([128, KC * NT], F16, tag="h1nTh", name="h1nTh")
            h1nTl = actp.tile([128, KC * NT], F16, tag="h1nTl", name="h1nTl")

            def gru_layer(layer, f, hTh, hTl, hnTh, hnTl, nh_dram):
                m_r, m_z, m_n = f, 8 + f, 16 + f
                if layer == 0:
                    wih_h, wih_l = whh0h_d, whh0l_d
                    brz, binb, bhnb = brz0_sb, bin0_sb, bhn0_sb
                else:
                    wih_h, wih_l = whh1h_d, whh1l_d
                    brz, binb, bhnb = brz1_sb, bin1_sb, bhn1_sb

                wrh = wmm.tile([128, H], F16, tag="w", name="wrh")
                nc.sync.dma_start(wrh[:], wih_h[m_r])
                wrl = wmm.tile([128, H], F16, tag="w", name="wrl")
                nc.sync.dma_start(wrl[:], wih_l[m_r])
                wzh = wmm.tile([128, H], F16, tag="w", name="wzh")
                nc.sync.dma_start(wzh[:], wih_h[m_z])
                wzl = wmm.tile([128, H], F16, tag="w", name="wzl")
                nc.sync.dma_start(wzl[:], wih_l[m_z])
                wnh = wmm.tile([128, H], F16, tag="w", name="wnh")
                nc.sync.dma_start(wnh[:], wih_h[m_n])
                wnl = wmm.tile([128, H], F16, tag="w", name="wnl")
                nc.sync.dma_start(wnl[:], wih_l[m_n])
                if layer == 1:
                    virh = wmm.tile([128, H], F16, tag="w", name="virh")
                    nc.sync.dma_start(virh[:], wih1h_d[m_r])
                    virl = wmm.tile([128, H], F16, tag="w", name="virl")
                    nc.sync.dma_start(virl[:], wih1l_d[m_r])
                    vizh = wmm.tile([128, H], F16, tag="w", name="vizh")
                    nc.sync.dma_start(vizh[:], wih1h_d[m_z])
                    vizl = wmm.tile([128, H], F16, tag="w", name="vizl")
                    nc.sync.dma_start(vizl[:], wih1l_d[m_z])
                    vinh = wmm.tile([128, H], F16, tag="w", name="vinh")
                    nc.sync.dma_start(vinh[:], wih1h_d[m_n])
                    vinl = wmm.tile([128, H], F16, tag="w", name="vinl")
                    nc.sync.dma_start(vinl[:], wih1l_d[m_n])

                def xpart(ps, mm, start):
                    # layer 0: x contribution (K=33); layer 1: h0_new contribution
                    if layer == 0:
                        sl = slice(mm * 128, (mm + 1) * 128)
                        mm3(ps, wih0h_sb[:, sl], wih0l_sb[:, sl], xTh[:], xTl[:],
                            start, False)
                    else:
                        w_h, w_l = {m_r: (virh, virl), m_z: (vizh, vizl),
                                    m_n: (vinh, vinl)}[mm]
                        for k in range(KC):
                            mm3(ps[:], w_h[:, k * 128:(k + 1) * 128],
                                w_l[:, k * 128:(k + 1) * 128],
                                h0nTh[:, k * NT:(k + 1) * NT],
                                h0nTl[:, k * NT:(k + 1) * NT],
                                start and k == 0, False)

                def hpart(ps, wh, wl, stop):
                    for k in range(KC):
                        mm3(ps[:], wh[:, k * 128:(k + 1) * 128],
                            wl[:, k * 128:(k + 1) * 128],
                            hTh[:, k * NT:(k + 1) * NT],
                            hTl[:, k * NT:(k + 1) * NT],
                            False, stop and k == KC - 1)

                ps_r = psmm.tile([128, NT], F32, tag="mm", name="ps_r")
                xpart(ps_r, m_r, True)
                hpart(ps_r, wrh, wrl, True)
                r_sb = ewp.tile([128, NT], F32, tag="ew", name="r_sb")
                nc.scalar.activation(r_sb[:], ps_r[:], AF.Sigmoid,
                                     bias=brz[:, f:f + 1])

                ps_z = psmm.tile([128, NT], F32, tag="mm", name="ps_z")
                xpart(ps_z, m_z, True)
                hpart(ps_z, wzh, wzl, True)
                z_sb = ewp.tile([128, NT], F32, tag="ew", name="z_sb")
                nc.scalar.activation(z_sb[:], ps_z[:], AF.Sigmoid,
                                     bias=brz[:, 8 + f:9 + f])

                ps_in = psmm.tile([128, NT], F32, tag="mm", name="ps_in")
                if layer == 0:
                    sl = slice(m_n * 128, (m_n + 1) * 128)
                    mm3(ps_in[:], wih0h_sb[:, sl], wih0l_sb[:, sl], xTh[:], xTl[:],
                        True, True)
                else:
                    for k in range(KC):
                        mm3(ps_in[:], vinh[:, k * 128:(k + 1) * 128],
                            vinl[:, k * 128:(k + 1) * 128],
                            h0nTh[:, k * NT:(k + 1) * NT],
                            h0nTl[:, k * NT:(k + 1) * NT],
                            k == 0, k == KC - 1)
                ps_hn = psmm.tile([128, NT], F32, tag="mm", name="ps_hn")
                for k in range(KC):
                    mm3(ps_hn[:], wnh[:, k * 128:(k + 1) * 128],
                        wnl[:, k * 128:(k + 1) * 128],
                        hTh[:, k * NT:(k + 1) * NT],
                        hTl[:, k * NT:(k + 1) * NT],
                        k == 0, k == KC - 1)

                t1 = ewp.tile([128, NT], F32, tag="ew", name="t1")
                nc.vector.scalar_tensor_tensor(t1[:], ps_hn[:], bhnb[:, f:f + 1],
                                               r_sb[:], OP.add, OP.mult)
                t2 = ewp.tile([128, NT], F32, tag="ew", name="t2")
                nc.vector.tensor_tensor(t2[:], t1[:], ps_in[:], OP.add)
                n_sb = ewp.tile([128, NT], F32, tag="ew", name="n_sb")
                nc.scalar.activation(n_sb[:], t2[:], AF.Tanh, bias=binb[:, f:f + 1])

                fsl = slice(f * NT, (f + 1) * NT)
                u_sb = ewp.tile([128, NT], F32, tag="ew", name="u_sb")
                nc.vector.tensor_tensor(u_sb[:], hTh[:, fsl], hTl[:, fsl], OP.add)
                d_sb = ewp.tile([128, NT], F32, tag="ew", name="d_sb")
                nc.vector.tensor_tensor(d_sb[:], u_sb[:], n_sb[:], OP.subtract)
                e_sb = ewp.tile([128, NT], F32, tag="ew", name="e_sb")
                nc.vector.tensor_tensor(e_sb[:], z_sb[:], d_sb[:], OP.mult)
                hnew = hnewp.tile([128, NT], F32, tag="hnew", name="hnew")
                nc.vector.tensor_tensor(hnew[:], n_sb[:], e_sb[:], OP.add)

                split_from(hnTh[:, fsl], hnTl[:, fsl], hnew[:])

                # transpose h_new back to batch-major; DMA each 128x128 piece
                # straight to DRAM (512B bursts, no staging buffer)
                for i in range(NT // 128):
                    ps = pstr.tile([128, 128], F32, tag="tr", name="ps_tr")
                    nc.tensor.transpose(ps[:], hnew[:, i * 128:(i + 1) * 128],
                                        ident_sb[:])
                    ot = otp.tile([128, 128], F32, tag="ot", name="ot")
                    pcopy(ot[:], ps[:])
                    nc.sync.dma_start(
                        nh_dram[b0 + i * 128:b0 + (i + 1) * 128,
                                f * 128:(f + 1) * 128], ot[:])

            # ---- layer 0 ----
            for f in range(KC):
                gru_layer(0, f, h0Th, h0Tl, h0nTh, h0nTl, nh0_d)

            # ---- layer 1 ----
            for f in range(KC):
                gru_layer(1, f, h1Th, h1Tl, h1nTh, h1nTl, nh1_d)

            # ---- heads (all gelus first: one ACT table-set switch each way) ----
            ep1T = ep1p.tile([128, 4 * NT], F32, tag="ep1T", name="ep1T")
            for mo in range(4):
                weh = wmm.tile([128, H], F16, tag="w", name="weh")
                nc.sync.dma_start(weh[:], epw1h_d[mo])
                wel = wmm.tile([128, H], F16, tag="w", name="wel")
                nc.sync.dma_start(wel[:], epw1l_d[mo])
                ps_e = psmm.tile([128, NT], F32, tag="mm", name="ps_e")
                for k in range(KC):
                    mm3(ps_e[:], weh[:, k * 128:(k + 1) * 128],
                        wel[:, k * 128:(k + 1) * 128],
                        h1nTh[:, k * NT:(k + 1) * NT],
                        h1nTl[:, k * NT:(k + 1) * NT],
                        k == 0, k == KC - 1)
                nc.scalar.activation(ep1T[:, mo * NT:(mo + 1) * NT], ps_e[:],
                                     AF.Gelu, bias=epb1_sb[:, mo:mo + 1])

            ps_p = psmm.tile([64, NT], F32, tag="mm", name="ps_p")
            for k in range(KC):
                mm3(ps_p[:], ppw1h_sb[:, k * 64:(k + 1) * 64],
                    ppw1l_sb[:, k * 64:(k + 1) * 64],
                    h1nTh[:, k * NT:(k + 1) * NT],
                    h1nTl[:, k * NT:(k + 1) * NT],
                    k == 0, k == KC - 1)
            pp1_sb = h64p.tile([64, NT], F32, tag="h64", name="pp1_sb")
            nc.scalar.activation(pp1_sb[:], ps_p[:], AF.Gelu, bias=ppb1_sb[:])

            ps_g = psmm.tile([64, NT], F32, tag="mm", name="ps_g")
            for k in range(KC):
                mm3(ps_g[:], sgw1h_sb[:, k * 64:(k + 1) * 64],
                    sgw1l_sb[:, k * 64:(k + 1) * 64],
                    h1nTh[:, k * NT:(k + 1) * NT],
                    h1nTl[:, k * NT:(k + 1) * NT],
                    k == 0, k == KC - 1)
            sg1_sb = h64p.tile([64, NT], F32, tag="h64", name="sg1_sb")
            nc.scalar.activation(sg1_sb[:], ps_g[:], AF.Gelu, bias=sgb1_sb[:])

            # energy head tail (fp32, M=1)
            ps_ev = psmm.tile([1, NT], F32, tag="mm", name="ps_ev")
            for mo in range(4):
                nc.tensor.matmul(ps_ev[:], epw2_sb[:, mo:mo + 1],
                                 ep1T[:, mo * NT:(mo + 1) * NT],
                                 start=(mo == 0), stop=(mo == 3))
            en_sb = rowp.tile([1, NT], F32, tag="rows", name="en_sb")
            nc.scalar.activation(en_sb[:], ps_ev[:], AF.Tanh, bias=epb2_sb[:])
            nc.sync.dma_start(en_d[0:1, b0:b0 + NT], en_sb[:])

            # position head tail: one M=1 matmul per coordinate (keeps every
            # row-op operand at partition base 0)
            pos_rows = []
            for c in range(3):
                ps_pc = psmm.tile([1, NT], F32, tag="mm", name="ps_pc")
                nc.tensor.matmul(ps_pc[:], ppw2_sb[:, c:c + 1], pp1_sb[:],
                                 start=True, stop=True)
                pn = rowp.tile([1, NT], F32, tag="rows", name=f"pn{c}")
                nc.scalar.activation(pn[:], ps_pc[:], AF.Tanh,
                                     bias=ppb2_sb[:, c:c + 1])
                # pred = ((pos+1)*0.5)*63  (first two ops exact, one rounding)
                s12 = rowp.tile([1, NT], F32, tag="rows", name=f"s12_{c}")
                nc.vector.tensor_scalar(s12[:], pn[:], 1.0, 0.5, OP.add, OP.mult)
                s3 = rowp.tile([1, NT], F32, tag="rows", name=f"s3_{c}")
                nc.vector.tensor_scalar(s3[:], s12[:], 63.0, None, OP.mult)
                pos_rows.append(s3)

            # spawn energy tail (fp16 3-pass, M=1)
            ps_se = psmm.tile([1, NT], F32, tag="mm", name="ps_se")
            for k in range(KC):
                mm3(ps_se[:], sewh_sb[:, k:k + 1], sewl_sb[:, k:k + 1],
                    h1nTh[:, k * NT:(k + 1) * NT],
                    h1nTl[:, k * NT:(k + 1) * NT],
                    k == 0, k == KC - 1)
            se_sb = rowp.tile([1, NT], F32, tag="rows", name="se_sb")
            nc.scalar.activation(se_sb[:], ps_se[:], AF.Tanh, bias=seb_sb[:])

            # spawn gate pre-activation (fp32, M=1)
            ps_g2 = psmm.tile([1, NT], F32, tag="mm", name="ps_g2")
            nc.tensor.matmul(ps_g2[:], sgw2_sb[:], sg1_sb[:], start=True, stop=True)
            g_sb = rowp.tile([1, NT], F32, tag="rows", name="g_sb")
            nc.vector.tensor_scalar(g_sb[:], ps_g2[:], sgb2_sb[:], None, OP.add)

            # flow-age z bias: 0.5 + fa*0.1
            fa_row = rowp.tile([1, NT], F32, tag="rows", name="fa_row")
            nc.sync.dma_start(fa_row[:], fa_d[0:1, b0:b0 + NT])
            faz = rowp.tile([1, NT], F32, tag="rows", name="faz")
            nc.vector.tensor_scalar(faz[:], fa_row[:], 0.1, 0.5, OP.mult, OP.add)
            pz = rowp.tile([1, NT], F32, tag="rows", name="pz")
            nc.vector.tensor_tensor(pz[:], pos_rows[2][:], faz[:], OP.add)

            # termination: oob_xy | (pz >= 32)   [reached|oob_z == pz>=32]
            cmp_or = None
            for c in range(2):
                lt = rowp.tile([1, NT], F32, tag="rows", name=f"lt{c}")
                nc.vector.tensor_scalar(lt[:], pos_rows[c][:], 0.0, None, OP.is_lt)
                ge = rowp.tile([1, NT], F32, tag="rows", name=f"ge{c}")
                nc.vector.tensor_scalar(ge[:], pos_rows[c][:], 64.0, None, OP.is_ge)
                oo = rowp.tile([1, NT], F32, tag="rows", name=f"oo{c}")
                nc.vector.tensor_tensor(oo[:], lt[:], ge[:], OP.logical_or)
                if cmp_or is None:
                    cmp_or = oo
                else:
                    oo2 = rowp.tile([1, NT], F32, tag="rows", name="oo2")
                    nc.vector.tensor_tensor(oo2[:], cmp_or[:], oo[:], OP.logical_or)
                    cmp_or = oo2
            gez = rowp.tile([1, NT], F32, tag="rows", name="gez")
            nc.vector.tensor_scalar(gez[:], pz[:], 32.0, None, OP.is_ge)
            term = rowp.tile([1, NT], F32, tag="rows", name="term")
            nc.vector.tensor_tensor(term[:], cmp_or[:], gez[:], OP.logical_or)
            nc.sync.dma_start(tm_d[0:1, b0:b0 + NT], term[:])

            # next_position: RNE round; z overridden to 32 where reached
            for c in range(2):
                a = rowp.tile([1, NT], F32, tag="rows", name=f"a{c}")
                nc.vector.tensor_scalar(a[:], pos_rows[c][:], MAGIC, None, OP.add)
                rr = rowp.tile([1, NT], F32, tag="rows", name=f"rr{c}")
                nc.vector.tensor_scalar(rr[:], a[:], MAGIC, None, OP.subtract)
                nc.sync.dma_start(pos_d[c:c + 1, b0:b0 + NT], rr[:])
            az = rowp.tile([1, NT], F32, tag="rows", name="az")
            nc.vector.tensor_scalar(az[:], pz[:], MAGIC, None, OP.add)
            rz = rowp.tile([1, NT], F32, tag="rows", name="rz")
            nc.vector.tensor_scalar(rz[:], az[:], MAGIC, None, OP.subtract)
            le63 = rowp.tile([1, NT], F32, tag="rows", name="le63")
            nc.vector.tensor_scalar(le63[:], pz[:], 63.0, None, OP.is_le)
            reach = rowp.tile([1, NT], F32, tag="rows", name="reach")
            nc.vector.tensor_tensor(reach[:], gez[:], le63[:], OP.mult)
            dd = rowp.tile([1, NT], F32, tag="rows", name="dd")
            nc.vector.tensor_scalar(dd[:], rz[:], 32.0, None, OP.subtract)
            ee = rowp.tile([1, NT], F32, tag="rows", name="ee")
            nc.vector.tensor_tensor(ee[:], reach[:], dd[:], OP.mult)
            npz = rowp.tile([1, NT], F32, tag="rows", name="npz")
            nc.vector.tensor_tensor(npz[:], rz[:], ee[:], OP.subtract)
            nc.sync.dma_start(pos_d[2:3, b0:b0 + NT], npz[:])

            # spawn counts from logit-space thresholds
            c1 = rowp.tile([1, NT], F32, tag="rows", name="c1")
            nc.vector.tensor_scalar(c1[:], g_sb[:], T0, None, OP.is_gt)
            c2 = rowp.tile([1, NT], F32, tag="rows", name="c2")
            nc.vector.tensor_scalar(c2[:], g_sb[:], T1, None, OP.is_gt)
            c3 = rowp.tile([1, NT], F32, tag="rows", name="c3")
            nc.vector.tensor_scalar(c3[:], g_sb[:], T2, None, OP.is_gt)
            s23 = rowp.tile([1, NT], F32, tag="rows", name="s23")
            nc.vector.tensor_tensor(s23[:], c2[:], c3[:], OP.add)
            s231 = rowp.tile([1, NT], F32, tag="rows", name="s231")
            nc.vector.tensor_scalar(s231[:], s23[:], 1.0, None, OP.add)
            cnt = rowp.tile([1, NT], F32, tag="rows", name="cnt")
            nc.vector.tensor_tensor(cnt[:], c1[:], s231[:], OP.mult)
            nc.sync.dma_start(ct_d[0:1, b0:b0 + NT], cnt[:])

            # spawn_frac = decided * se / (counts+1)
            cp1 = rowp.tile([1, NT], F32, tag="rows", name="cp1")
            nc.vector.tensor_scalar(cp1[:], cnt[:], 1.0, None, OP.add)
            # DVE has no divide op; reciprocal is exact for divisors 1/2/4 and
            # correctly rounded for 3 (<=1ulp off true division on frac scale)
            rec = rowp.tile([1, NT], F32, tag="rows", name="rec")
            nc.vector.reciprocal(rec[:], cp1[:])
            q = rowp.tile([1, NT], F32, tag="rows", name="q")
            nc.vector.tensor_tensor(q[:], se_sb[:], rec[:], OP.mult)
            frac = rowp.tile([1, NT], F32, tag="rows", name="frac")
            nc.vector.tensor_tensor(frac[:], c1[:], q[:], OP.mult)
            nc.sync.dma_start(fr_d[0:1, b0:b0 + NT], frac[:])

    nc.compile()
    return nc


def _get_prog():
    global _PROG
    if _PROG is None:
        _PROG = _build()
    return _PROG


def kernel(neuron_output, embedding_part, hidden_state, flow_age,
           w_ih0, w_hh0, b_ih0, b_hh0, w_ih1, w_hh1, b_ih1, b_hh1,
           ep_w1, ep_b1, ep_w2, ep_b2, pp_w1, pp_b1, pp_w2, pp_b2,
           sg_w1, sg_b1, sg_w2, sg_b2, se_w, se_b):
    global LAST_RES
    nc = _get_prog()

    f32 = np.float32
    x_full = np.concatenate([np.asarray(neuron_output, f32),
                             np.asarray(embedding_part, f32)], axis=1)
    hs = np.asarray(hidden_state, f32)
    fa_full = np.asarray(flow_age, f32)

    wih0t = np.ascontiguousarray(np.asarray(w_ih0, f32).T)
    whh0p = _pack_big(w_hh0)
    wih1p = _pack_big(w_ih1)
    whh1p = _pack_big(w_hh1)
    epw1p = _pack_big(ep_w1)
    ppw1t = _pack_kx(pp_w1)
    sgw1t = _pack_kx(sg_w1)
    sewt = _pack_kx(se_w)

    shared = {
        "ident": np.eye(128, dtype=f32),
        "wih0h": _hi(wih0t), "wih0l": _lo(wih0t),
        "whh0h": _hi(whh0p), "whh0l": _lo(whh0p),
        "wih1h": _hi(wih1p), "wih1l": _lo(wih1p),
        "whh1h": _hi(whh1p), "whh1l": _lo(whh1p),
        "epw1h": _hi(epw1p), "epw1l": _lo(epw1p),
        "epw2t": _pack_kx(ep_w2),
        "ppw1h": _hi(ppw1t), "ppw1l": _lo(ppw1t),
        "ppw2t": np.ascontiguousarray(np.asarray(pp_w2, f32).T),
        "sgw1h": _hi(sgw1t), "sgw1l": _lo(sgw1t),
        "sgw2t": np.ascontiguousarray(np.asarray(sg_w2, f32).T),
        "sewh": _hi(sewt), "sewl": _lo(sewt),
        "brz0": np.ascontiguousarray(
            (np.asarray(b_ih0, f32) + np.asarray(b_hh0, f32))[:2 * H]
            .reshape(16, 128).T),
        "bin0": np.ascontiguousarray(np.asarray(b_ih0, f32)[2 * H:].reshape(8, 128).T),
        "bhn0": np.ascontiguousarray(np.asarray(b_hh0, f32)[2 * H:].reshape(8, 128).T),
        "brz1": np.ascontiguousarray(
            (np.asarray(b_ih1, f32) + np.asarray(b_hh1, f32))[:2 * H]
            .reshape(16, 128).T),
        "bin1": np.ascontiguousarray(np.asarray(b_ih1, f32)[2 * H:].reshape(8, 128).T),
        "bhn1": np.ascontiguousarray(np.asarray(b_hh1, f32)[2 * H:].reshape(8, 128).T),
        "epb1": np.ascontiguousarray(np.asarray(ep_b1, f32).reshape(4, 128).T),
        "ppb1": np.asarray(pp_b1, f32).reshape(64, 1),
        "sgb1": np.asarray(sg_b1, f32).reshape(64, 1),
        "epb2": np.asarray(ep_b2, f32).reshape(1, 1),
        "ppb2": np.asarray(pp_b2, f32).reshape(1, 3),
        "sgb2": np.asarray(sg_b2, f32).reshape(1, 1),
        "seb": np.asarray(se_b, f32).reshape(1, 1),
    }

    in_maps = []
    for c in range(CORES):
        sl = slice(c * BS, (c + 1) * BS)
        m = dict(shared)
        m["x"] = np.ascontiguousarray(x_full[sl])
        for li, key in ((0, "h0"), (1, "h1")):
            hsl = np.ascontiguousarray(hs[li, sl])
            hi16 = hsl.astype(np.float16)
            m[key + "hi"] = hi16
            m[key + "lo"] = (hsl - hi16.astype(np.float32)).astype(np.float16)
        m["fa"] = np.ascontiguousarray(fa_full[sl].reshape(1, BS))
        in_maps.append(m)

    trace = bool(os.environ.get("EC_TRACE"))
    res = run_bass_kernel_spmd(nc, in_maps, list(range(CORES)), trace=trace)
    LAST_RES = res

    energy = np.concatenate([res.results[c]["energy"].reshape(BS, 1)
                             for c in range(CORES)], axis=0)
    pos = np.concatenate([np.ascontiguousarray(res.results[c]["pos"].T)
                          for c in range(CORES)], axis=0)
    frac = np.concatenate([res.results[c]["frac"].reshape(BS)
                           for c in range(CORES)], axis=0)
    counts = np.concatenate([res.results[c]["counts"].reshape(BS)
                             for c in range(CORES)], axis=0).astype(np.int32)
    term = (np.concatenate([res.results[c]["term"].reshape(BS)
                            for c in range(CORES)], axis=0) > 0.5)
    nh = np.stack([
        np.concatenate([res.results[c]["nh0"] for c in range(CORES)], axis=0),
        np.concatenate([res.results[c]["nh1"] for c in range(CORES)], axis=0),
    ], axis=0)
    return energy, pos, frac, counts, term, nh


# revision 14
# speedup vs baseline: 1.0762x; 1.0762x over previous
"""EnergyCarrier (2-layer GRU cell + heads) Trainium2 kernel.

Full inputs in, full outputs out. Data-parallel over 8 NeuronCores:
batch dim B=32768 sharded into 8x4096 rows; GRU/head weights replicated.

On-chip layout is feature-major ([feature_chunk=128 partitions, batch free])
so the GRU matmuls contract over partitions; activations are transposed at
the DRAM boundary with PE-mode transposes.

Precision scheme: the outputs include discrete quantities (is_terminated
bool, spawn_counts int, rounded positions) whose boundary flips dominate
the error budget, so bf16/f32r matmuls (~2^-12 operand truncation) are out.
Instead every large matmul runs as a 3-pass fp16 hi/lo split
(hi.hi + hi.lo + lo.hi accumulated in one PSUM bank): fp16 products are
exact in the PE's e10m23 accumulator and the dropped lo.lo term is
~2^-22 relative, giving fp32-grade results (measured ~8e-7 rel on HW) at
3 cycles/row instead of fp32's 4. Tiny M<=3 matmuls stay plain fp32.

Spawn decisions are computed from the spawn-gate pre-activation (thresholds
0, ln2, ln5 in logit space) instead of the sigmoid output: the ACT sigmoid
table has a 40-ULP budget, while the matmul pre-activation is fp32-grade.
Rounding uses the +/-1.5*2^23 magic-number trick (RNE, matches jnp.round's
half-to-even).
"""

import os
import sys
from contextlib import ExitStack

sys.path.insert(0, "/opt/trn_rl_repo")

import numpy as np

import concourse.bass as bass
import concourse.tile as tile
from concourse import bacc, mybir
from concourse.bass_utils import run_bass_kernel_spmd

AF = mybir.ActivationFunctionType
OP = mybir.AluOpType
F32 = mybir.dt.float32
F16 = mybir.dt.float16

B, H, IN = 32768, 1024, 33
CORES = 8
BS = B // CORES          # rows per core
NT = 512                 # batch-tile columns
NBT = BS // NT
KC = H // 128            # feature chunks
MAGIC = 12582912.0       # 1.5 * 2**23 -> RNE round-to-integer via add/sub
# spawn thresholds in pre-activation (logit) space:
# decided: sigmoid32(g) > 0.5 <=> g > 2^-23 (fp32 rounding boundary near 0.5)
T0 = float(np.float32(2.0 ** -23))
T1 = float(np.float32(np.log(2.0)))   # p > 2/3
T2 = float(np.float32(np.log(5.0)))   # p > 5/6

LAST_RES = None  # BassKernelResults of the most recent run (for test harness)
_PROG = None


def _pack_big(w):
    """[O,K] (O,K mult of 128) -> [O/128, 128, K]; [m,p,k*128+j] = w[m*128+j, k*128+p]."""
    o, k = w.shape
    m, kc = o // 128, k // 128
    w4 = np.asarray(w, np.float32).reshape(m, 128, kc, 128)
    return np.ascontiguousarray(w4.transpose(0, 3, 2, 1).reshape(m, 128, k))


def _pack_kx(w):
    """[O<=128, K] -> [128, (K/128)*O]; [p, k*O+j] = w[j, k*128+p]."""
    o, k = w.shape
    kc = k // 128
    w3 = np.asarray(w, np.float32).reshape(o, kc, 128)
    return np.ascontiguousarray(w3.transpose(2, 1, 0).reshape(128, kc * o))


def _hi(a):
    return np.asarray(a, np.float32).astype(np.float16)


def _lo(a):
    a = np.asarray(a, np.float32)
    return (a - a.astype(np.float16).astype(np.float32)).astype(np.float16)


def _build():
    nc = bacc.Bacc("TRN2", target_bir_lowering=False, debug=False,
                   num_devices=CORES)

    def din(name, shape, dt=F32):
        return nc.dram_tensor(name, list(shape), dt, kind="ExternalInput").ap()

    def dout(name, shape):
        return nc.dram_tensor(name, list(shape), F32, kind="ExternalOutput").ap()

    x_d = din("x", (BS, IN))
    h0hi_d = din("h0hi", (BS, H), F16)
    h0lo_d = din("h0lo", (BS, H), F16)
    h1hi_d = din("h1hi", (BS, H), F16)
    h1lo_d = din("h1lo", (BS, H), F16)
    fa_d = din("fa", (1, BS))
    ident_d = din("ident", (128, 128))
    wih0h_d = din("wih0h", (IN, 3 * H), F16)
    wih0l_d = din("wih0l", (IN, 3 * H), F16)
    whh0h_d = din("whh0h", (24, 128, H), F16)
    whh0l_d = din("whh0l", (24, 128, H), F16)
    wih1h_d = din("wih1h", (24, 128, H), F16)
    wih1l_d = din("wih1l", (24, 128, H), F16)
    whh1h_d = din("whh1h", (24, 128, H), F16)
    whh1l_d = din("whh1l", (24, 128, H), F16)
    epw1h_d = din("epw1h", (4, 128, H), F16)
    epw1l_d = din("epw1l", (4, 128, H), F16)
    epw2_d = din("epw2t", (128, 4))
    ppw1h_d = din("ppw1h", (128, KC * 64), F16)
    ppw1l_d = din("ppw1l", (128, KC * 64), F16)
    ppw2_d = din("ppw2t", (64, 3))
    sgw1h_d = din("sgw1h", (128, KC * 64), F16)
    sgw1l_d = din("sgw1l", (128, KC * 64), F16)
    sgw2_d = din("sgw2t", (64, 1))
    sewh_d = din("sewh", (128, KC), F16)
    sewl_d = din("sewl", (128, KC), F16)
    brz0_d = din("brz0", (128, 16))
    bin0_d = din("bin0", (128, 8))
    bhn0_d = din("bhn0", (128, 8))
    brz1_d = din("brz1", (128, 16))
    bin1_d = din("bin1", (128, 8))
    bhn1_d = din("bhn1", (128, 8))
    epb1_d = din("epb1", (128, 4))
    ppb1_d = din("ppb1", (64, 1))
    sgb1_d = din("sgb1", (64, 1))
    epb2_d = din("epb2", (1, 1))
    ppb2_d = din("ppb2", (1, 3))
    sgb2_d = din("sgb2", (1, 1))
    seb_d = din("seb", (1, 1))

    en_d = dout("energy", (1, BS))
    pos_d = dout("pos", (3, BS))
    fr_d = dout("frac", (1, BS))
    ct_d = dout("counts", (1, BS))
    tm_d = dout("term", (1, BS))
    nh0_d = dout("nh0", (BS, H))
    nh1_d = dout("nh1", (BS, H))

    with tile.TileContext(nc) as tc, ExitStack() as ctx:
        const = ctx.enter_context(tc.tile_pool(name="const", bufs=1))
        hload = ctx.enter_context(tc.tile_pool(name="hload", bufs=3))
        wmm = ctx.enter_context(tc.tile_pool(name="wmm", bufs=14))
        actp = ctx.enter_context(tc.tile_pool(name="actp", bufs=1))
        ewp = ctx.enter_context(tc.tile_pool(name="ewp", bufs=9))
        hnewp = ctx.enter_context(tc.tile_pool(name="hnewp", bufs=3))
        rowp = ctx.enter_context(tc.tile_pool(name="rowp", bufs=10))
        h64p = ctx.enter_context(tc.tile_pool(name="h64p", bufs=4))
        otp = ctx.enter_context(tc.tile_pool(name="otp", bufs=6))
        xtp = ctx.enter_context(tc.tile_pool(name="xtp", bufs=1))
        ep1p = ctx.enter_context(tc.tile_pool(name="ep1p", bufs=1))
        psmm = ctx.enter_context(tc.tile_pool(name="psmm", bufs=6, space="PSUM"))
        pstr = ctx.enter_context(tc.tile_pool(name="pstr", bufs=2, space="PSUM"))

        def ctile(ap_dram, shape, name, dt=F32):
            t = const.tile(shape, dt, tag=name, name=name)
            nc.sync.dma_start(t[:], ap_dram[:])
            return t

        ident_sb = ctile(ident_d, [128, 128], "ident_sb")
        wih0h_sb = ctile(wih0h_d, [IN, 3 * H], "wih0h_sb", F16)
        wih0l_sb = ctile(wih0l_d, [IN, 3 * H], "wih0l_sb", F16)
        ppw1h_sb = ctile(ppw1h_d, [128, KC * 64], "ppw1h_sb", F16)
        ppw1l_sb = ctile(ppw1l_d, [128, KC * 64], "ppw1l_sb", F16)
        sgw1h_sb = ctile(sgw1h_d, [128, KC * 64], "sgw1h_sb", F16)
        sgw1l_sb = ctile(sgw1l_d, [128, KC * 64], "sgw1l_sb", F16)
        sewh_sb = ctile(sewh_d, [128, KC], "sewh_sb", F16)
        sewl_sb = ctile(sewl_d, [128, KC], "sewl_sb", F16)
        epw2_sb = ctile(epw2_d, [128, 4], "epw2_sb")
        ppw2_sb = ctile(ppw2_d, [64, 3], "ppw2_sb")
        sgw2_sb = ctile(sgw2_d, [64, 1], "sgw2_sb")
        brz0_sb = ctile(brz0_d, [128, 16], "brz0_sb")
        bin0_sb = ctile(bin0_d, [128, 8], "bin0_sb")
        bhn0_sb = ctile(bhn0_d, [128, 8], "bhn0_sb")
        brz1_sb = ctile(brz1_d, [128, 16], "brz1_sb")
        bin1_sb = ctile(bin1_d, [128, 8], "bin1_sb")
        bhn1_sb = ctile(bhn1_d, [128, 8], "bhn1_sb")
        epb1_sb = ctile(epb1_d, [128, 4], "epb1_sb")
        ppb1_sb = ctile(ppb1_d, [64, 1], "ppb1_sb")
        sgb1_sb = ctile(sgb1_d, [64, 1], "sgb1_sb")
        epb2_sb = ctile(epb2_d, [1, 1], "epb2_sb")
        ppb2_sb = ctile(ppb2_d, [1, 3], "ppb2_sb")
        sgb2_sb = ctile(sgb2_d, [1, 1], "sgb2_sb")
        seb_sb = ctile(seb_d, [1, 1], "seb_sb")

        copy_ctr = [0]

        def pcopy(dst, src):
            # alternate DVE/ACT so neither engine owns all psum drains
            if copy_ctr[0] % 2 == 0:
                nc.vector.tensor_copy(dst, src)
            else:
                nc.scalar.copy(dst, src)
            copy_ctr[0] += 1

        def split_from(dst_hi, dst_lo, src):
            # hi = fp16(src); lo = fp16(src - hi). src may be PSUM (1 psum
            # operand per DVE op). Exact to ~2^-22 regardless of rounding.
            pcopy(dst_hi, src)
            nc.vector.scalar_tensor_tensor(dst_lo, dst_hi, -1.0, src,
                                           OP.mult, OP.add)

        def load_transposed(dst_hi, dst_lo, hi_dram, lo_dram, b0):
            # dst[p, k*NT + j] <- src[b0+j, 128k+p]; host pre-split fp16 hi/lo,
            # loaded feature-major via hardware DMA transpose (2-byte dtype)
            for k in range(KC):
                nc.sync.dma_start_transpose(
                    dst_hi[:, k * NT:(k + 1) * NT],
                    hi_dram[b0:b0 + NT, k * 128:(k + 1) * 128])
                nc.sync.dma_start_transpose(
                    dst_lo[:, k * NT:(k + 1) * NT],
                    lo_dram[b0:b0 + NT, k * 128:(k + 1) * 128])

        def mm3(ps, wh, wl, xh, xl, start, stop):
            nc.tensor.matmul(ps, wh, xh, start=start, stop=False)
            nc.tensor.matmul(ps, wh, xl, start=False, stop=False)
            nc.tensor.matmul(ps, wl, xh, start=False, stop=stop)

        for bt in range(NBT):
            b0 = bt * NT

            # ---- inputs: h_prev^T then x^T (fp16 hi/lo pairs) ----
            h0Th = actp.tile([128, KC * NT], F16, tag="h0Th", name="h0Th",
                             bufs=2)
            h0Tl = actp.tile([128, KC * NT], F16, tag="h0Tl", name="h0Tl",
                             bufs=2)
            load_transposed(h0Th, h0Tl, h0hi_d, h0lo_d, b0)
            h1Th = actp.tile([128, KC * NT], F16, tag="h1Th", name="h1Th",
                             bufs=2)
            h1Tl = actp.tile([128, KC * NT], F16, tag="h1Tl", name="h1Tl",
                             bufs=2)
            load_transposed(h1Th, h1Tl, h1hi_d, h1lo_d, b0)

            xTh = xtp.tile([IN, NT], F16, tag="xTh", name="xTh")
            xTl = xtp.tile([IN, NT], F16, tag="xTl", name="xTl")
            for i in range(NT // 128):
                xb = hload.tile([128, IN], F32, tag="xb", name="xb")
                nc.sync.dma_start(xb[:], x_d[b0 + i * 128:b0 + (i + 1) * 128, :])
                psx = pstr.tile([128, 128], F32, tag="tr", name="ps_tr")
                nc.tensor.transpose(psx[0:IN, :], xb[:], ident_sb[:])
                sl = slice(i * 128, (i + 1) * 128)
                split_from(xTh[:, sl], xTl[:, sl], psx[0:IN, :])

            h0nTh = actp.tile([128, KC * NT], F16, tag="h0nTh", name="h0nTh")
            h0nTl = actp.tile([128, KC * NT], F16, tag="h0nTl", name="h0nTl")
            h1nTh = actp.tile([128, KC * NT], F16, tag="h1nTh", name="h1nTh")
            h1nTl = actp.tile([128, KC * NT], F16, tag="h1nTl", name="h1nTl")

            def gru_layer(layer, f, hTh, hTl, hnTh, hnTl, nh_dram):
                m_r, m_z, m_n = f, 8 + f, 16 + f
                if layer == 0:
                    wih_h, wih_l = whh0h_d, whh0l_d
                    brz, binb, bhnb = brz0_sb, bin0_sb, bhn0_sb
                else:
                    wih_h, wih_l = whh1h_d, whh1l_d
                    brz, binb, bhnb = brz1_sb, bin1_sb, bhn1_sb

                wrh = wmm.tile([128, H], F16, tag="w", name="wrh")
                nc.sync.dma_start(wrh[:], wih_h[m_r])
                wrl = wmm.tile([128, H], F16, tag="w", name="wrl")
                nc.sync.dma_start(wrl[:], wih_l[m_r])
                wzh = wmm.tile([128, H], F16, tag="w", name="wzh")
                nc.sync.dma_start(wzh[:], wih_h[m_z])
                wzl = wmm.tile([128, H], F16, tag="w", name="wzl")
                nc.sync.dma_start(wzl[:], wih_l[m_z])
                wnh = wmm.tile([128, H], F16, tag="w", name="wnh")
                nc.sync.dma_start(wnh[:], wih_h[m_n])
                wnl = wmm.tile([128, H], F16, tag="w", name="wnl")
                nc.sync.dma_start(wnl[:], wih_l[m_n])
                if layer == 1:
                    virh = wmm.tile([128, H], F16, tag="w", name="virh")
                    nc.sync.dma_start(virh[:], wih1h_d[m_r])
                    virl = wmm.tile([128, H], F16, tag="w", name="virl")
                    nc.sync.dma_start(virl[:], wih1l_d[m_r])
                    vizh = wmm.tile([128, H], F16, tag="w", name="vizh")
                    nc.sync.dma_start(vizh[:], wih1h_d[m_z])
                    vizl = wmm.tile([128, H], F16, tag="w", name="vizl")
                    nc.sync.dma_start(vizl[:], wih1l_d[m_z])
                    vinh = wmm.tile([128, H], F16, tag="w", name="vinh")
                    nc.sync.dma_start(vinh[:], wih1h_d[m_n])
                    vinl = wmm.tile([128, H], F16, tag="w", name="vinl")
                    nc.sync.dma_start(vinl[:], wih1l_d[m_n])

                def xpart(ps, mm, start):
                    # layer 0: x contribution (K=33); layer 1: h0_new contribution
                    if layer == 0:
                        sl = slice(mm * 128, (mm + 1) * 128)
                        mm3(ps, wih0h_sb[:, sl], wih0l_sb[:, sl], xTh[:], xTl[:],
                            start, False)
                    else:
                        w_h, w_l = {m_r: (virh, virl), m_z: (vizh, vizl),
                                    m_n: (vinh, vinl)}[mm]
                        for k in range(KC):
                            mm3(ps[:], w_h[:, k * 128:(k + 1) * 128],
                                w_l[:, k * 128:(k + 1) * 128],
                                h0nTh[:, k * NT:(k + 1) * NT],
                                h0nTl[:, k * NT:(k + 1) * NT],
                                start and k == 0, False)

                def hpart(ps, wh, wl, stop):
                    for k in range(KC):
                        mm3(ps[:], wh[:, k * 128:(k + 1) * 128],
                            wl[:, k * 128:(k + 1) * 128],
                            hTh[:, k * NT:(k + 1) * NT],
                            hTl[:, k * NT:(k + 1) * NT],
                            False, stop and k == KC - 1)

                ps_r = psmm.tile([128, NT], F32, tag="mm", name="ps_r")
                xpart(ps_r, m_r, True)
                hpart(ps_r, wrh, wrl, True)
                r_sb = ewp.tile([128, NT], F32, tag="ew", name="r_sb")
                nc.scalar.activation(r_sb[:], ps_r[:], AF.Sigmoid,
                                     bias=brz[:, f:f + 1])

                ps_z = psmm.tile([128, NT], F32, tag="mm", name="ps_z")
                xpart(ps_z, m_z, True)
                hpart(ps_z, wzh, wzl, True)
                z_sb = ewp.tile([128, NT], F32, tag="ew", name="z_sb")
                nc.scalar.activation(z_sb[:], ps_z[:], AF.Sigmoid,
                                     bias=brz[:, 8 + f:9 + f])

                ps_in = psmm.tile([128, NT], F32, tag="mm", name="ps_in")
                if layer == 0:
                    sl = slice(m_n * 128, (m_n + 1) * 128)
                    mm3(ps_in[:], wih0h_sb[:, sl], wih0l_sb[:, sl], xTh[:], xTl[:],
                        True, True)
                else:
                    for k in range(KC):
                        mm3(ps_in[:], vinh[:, k * 128:(k + 1) * 128],
                            vinl[:, k * 128:(k + 1) * 128],
                            h0nTh[:, k * NT:(k + 1) * NT],
                            h0nTl[:, k * NT:(k + 1) * NT],
                            k == 0, k == KC - 1)
                ps_hn = psmm.tile([128, NT], F32, tag="mm", name="ps_hn")
                for k in range(KC):
                    mm3(ps_hn[:], wnh[:, k * 128:(k + 1) * 128],
                        wnl[:, k * 128:(k + 1) * 128],
                        hTh[:, k * NT:(k + 1) * NT],
                        hTl[:, k * NT:(k + 1) * NT],
                        k == 0, k == KC - 1)

                t1 = ewp.tile([128, NT], F32, tag="ew", name="t1")
                nc.vector.scalar_tensor_tensor(t1[:], ps_hn[:], bhnb[:, f:f + 1],
                                               r_sb[:], OP.add, OP.mult)
                t2 = ewp.tile([128, NT], F32, tag="ew", name="t2")
                nc.vector.tensor_tensor(t2[:], t1[:], ps_in[:], OP.add)
                n_sb = ewp.tile([128, NT], F32, tag="ew", name="n_sb")
                nc.scalar.activation(n_sb[:], t2[:], AF.Tanh, bias=binb[:, f:f + 1])

                fsl = slice(f * NT, (f + 1) * NT)
                u_sb = ewp.tile([128, NT], F32, tag="ew", name="u_sb")
                nc.vector.tensor_tensor(u_sb[:], hTh[:, fsl], hTl[:, fsl], OP.add)
                d_sb = ewp.tile([128, NT], F32, tag="ew", name="d_sb")
                nc.vector.tensor_tensor(d_sb[:], u_sb[:], n_sb[:], OP.subtract)
                e_sb = ewp.tile([128, NT], F32, tag="ew", name="e_sb")
                nc.vector.tensor_tensor(e_sb[:], z_sb[:], d_sb[:], OP.mult)
                hnew = hnewp.tile([128, NT], F32, tag="hnew", name="hnew")
                nc.vector.tensor_tensor(hnew[:], n_sb[:], e_sb[:], OP.add)

                split_from(hnTh[:, fsl], hnTl[:, fsl], hnew[:])

                # transpose h_new back to batch-major; DMA each 128x128 piece
                # straight to DRAM (512B bursts, no staging buffer)
                for i in range(NT // 128):
                    ps = pstr.tile([128, 128], F32, tag="tr", name="ps_tr")
                    nc.tensor.transpose(ps[:], hnew[:, i * 128:(i + 1) * 128],
                                        ident_sb[:])
                    ot = otp.tile([128, 128], F32, tag="ot", name="ot")
                    pcopy(ot[:], ps[:])
                    nc.sync.dma_start(
                        nh_dram[b0 + i * 128:b0 + (i + 1) * 128,
                                f * 128:(f + 1) * 128], ot[:])

            # ---- layer 0 ----
            for f in range(KC):
                gru_layer(0, f, h0Th, h0Tl, h0nTh, h0nTl, nh0_d)

            # ---- layer 1 ----
            for f in range(KC):
                gru_layer(1, f, h1Th, h1Tl, h1nTh, h1nTl, nh1_d)

            # ---- heads (all gelus first: one ACT table-set switch each way) ----
            ep1T = ep1p.tile([128, 4 * NT], F32, tag="ep1T", name="ep1T")
            for mo in range(4):
                weh = wmm.tile([128, H], F16, tag="w", name="weh")
                nc.sync.dma_start(weh[:], epw1h_d[mo])
                wel = wmm.tile([128, H], F16, tag="w", name="wel")
                nc.sync.dma_start(wel[:], epw1l_d[mo])
                ps_e = psmm.tile([128, NT], F32, tag="mm", name="ps_e")
                for k in range(KC):
                    mm3(ps_e[:], weh[:, k * 128:(k + 1) * 128],
                        wel[:, k * 128:(k + 1) * 128],
                        h1nTh[:, k * NT:(k + 1) * NT],
                        h1nTl[:, k * NT:(k + 1) * NT],
                        k == 0, k == KC - 1)
                nc.scalar.activation(ep1T[:, mo * NT:(mo + 1) * NT], ps_e[:],
                                     AF.Gelu, bias=epb1_sb[:, mo:mo + 1])

            ps_p = psmm.tile([64, NT], F32, tag="mm", name="ps_p")
            for k in range(KC):
                mm3(ps_p[:], ppw1h_sb[:, k * 64:(k + 1) * 64],
                    ppw1l_sb[:, k * 64:(k + 1) * 64],
                    h1nTh[:, k * NT:(k + 1) * NT],
                    h1nTl[:, k * NT:(k + 1) * NT],
                    k == 0, k == KC - 1)
            pp1_sb = h64p.tile([64, NT], F32, tag="h64", name="pp1_sb")
            nc.scalar.activation(pp1_sb[:], ps_p[:], AF.Gelu, bias=ppb1_sb[:])

            ps_g = psmm.tile([64, NT], F32, tag="mm", name="ps_g")
            for k in range(KC):
                mm3(ps_g[:], sgw1h_sb[:, k * 64:(k + 1) * 64],
                    sgw1l_sb[:, k * 64:(k + 1) * 64],
                    h1nTh[:, k * NT:(k + 1) * NT],
                    h1nTl[:, k * NT:(k + 1) * NT],
                    k == 0, k == KC - 1)
            sg1_sb = h64p.tile([64, NT], F32, tag="h64", name="sg1_sb")
            nc.scalar.activation(sg1_sb[:], ps_g[:], AF.Gelu, bias=sgb1_sb[:])

            # energy head tail (fp32, M=1)
            ps_ev = psmm.tile([1, NT], F32, tag="mm", name="ps_ev")
            for mo in range(4):
                nc.tensor.matmul(ps_ev[:], epw2_sb[:, mo:mo + 1],
                                 ep1T[:, mo * NT:(mo + 1) * NT],
                                 start=(mo == 0), stop=(mo == 3))
            en_sb = rowp.tile([1, NT], F32, tag="rows", name="en_sb")
            nc.scalar.activation(en_sb[:], ps_ev[:], AF.Tanh, bias=epb2_sb[:])
            nc.sync.dma_start(en_d[0:1, b0:b0 + NT], en_sb[:])

            # position head tail: one M=1 matmul per coordinate (keeps every
            # row-op operand at partition base 0)
            pos_rows = []
            for c in range(3):
                ps_pc = psmm.tile([1, NT], F32, tag="mm", name="ps_pc")
                nc.tensor.matmul(ps_pc[:], ppw2_sb[:, c:c + 1], pp1_sb[:],
                                 start=True, stop=True)
                pn = rowp.tile([1, NT], F32, tag="rows", name=f"pn{c}")
                nc.scalar.activation(pn[:], ps_pc[:], AF.Tanh,
                                     bias=ppb2_sb[:, c:c + 1])
                # pred = ((pos+1)*0.5)*63  (first two ops exact, one rounding)
                s12 = rowp.tile([1, NT], F32, tag="rows", name=f"s12_{c}")
                nc.vector.tensor_scalar(s12[:], pn[:], 1.0, 0.5, OP.add, OP.mult)
                s3 = rowp.tile([1, NT], F32, tag="rows", name=f"s3_{c}")
                nc.vector.tensor_scalar(s3[:], s12[:], 63.0, None, OP.mult)
                pos_rows.append(s3)

            # spawn energy tail (fp16 3-pass, M=1)
            ps_se = psmm.tile([1, NT], F32, tag="mm", name="ps_se")
            for k in range(KC):
                mm3(ps_se[:], sewh_sb[:, k:k + 1], sewl_sb[:, k:k + 1],
                    h1nTh[:, k * NT:(k + 1) * NT],
                    h1nTl[:, k * NT:(k + 1) * NT],
                    k == 0, k == KC - 1)
            se_sb = rowp.tile([1, NT], F32, tag="rows", name="se_sb")
            nc.scalar.activation(se_sb[:], ps_se[:], AF.Tanh, bias=seb_sb[:])

            # spawn gate pre-activation (fp32, M=1)
            ps_g2 = psmm.tile([1, NT], F32, tag="mm", name="ps_g2")
            nc.tensor.matmul(ps_g2[:], sgw2_sb[:], sg1_sb[:], start=True, stop=True)
            g_sb = rowp.tile([1, NT], F32, tag="rows", name="g_sb")
            nc.vector.tensor_scalar(g_sb[:], ps_g2[:], sgb2_sb[:], None, OP.add)

            # flow-age z bias: 0.5 + fa*0.1
            fa_row = rowp.tile([1, NT], F32, tag="rows", name="fa_row")
            nc.sync.dma_start(fa_row[:], fa_d[0:1, b0:b0 + NT])
            faz = rowp.tile([1, NT], F32, tag="rows", name="faz")
            nc.vector.tensor_scalar(faz[:], fa_row[:], 0.1, 0.5, OP.mult, OP.add)
            pz = rowp.tile([1, NT], F32, tag="rows", name="pz")
            nc.vector.tensor_tensor(pz[:], pos_rows[2][:], faz[:], OP.add)

            # termination: oob_xy | (pz >= 32)   [reached|oob_z == pz>=32]
            cmp_or = None
            for c in range(2):
                lt = rowp.tile([1, NT], F32, tag="rows", name=f"lt{c}")
                nc.vector.tensor_scalar(lt[:], pos_rows[c][:], 0.0, None, OP.is_lt)
                ge = rowp.tile([1, NT], F32, tag="rows", name=f"ge{c}")
                nc.vector.tensor_scalar(ge[:], pos_rows[c][:], 64.0, None, OP.is_ge)
                oo = rowp.tile([1, NT], F32, tag="rows", name=f"oo{c}")
                nc.vector.tensor_tensor(oo[:], lt[:], ge[:], OP.logical_or)
                if cmp_or is None:
                    cmp_or = oo
                else:
                    oo2 = rowp.tile([1, NT], F32, tag="rows", name="oo2")
                    nc.vector.tensor_tensor(oo2[:], cmp_or[:], oo[:], OP.logical_or)
                    cmp_or = oo2
            gez = rowp.tile([1, NT], F32, tag="rows", name="gez")
            nc.vector.tensor_scalar(gez[:], pz[:], 32.0, None, OP.is_ge)
            term = rowp.tile([1, NT], F32, tag="rows", name="term")
            nc.vector.tensor_tensor(term[:], cmp_or[:], gez[:], OP.logical_or)
            nc.sync.dma_start(tm_d[0:1, b0:b0 + NT], term[:])

            # next_position: RNE round; z overridden to 32 where reached
            for c in range(2):
                a = rowp.tile([1, NT], F32, tag="rows", name=f"a{c}")
                nc.vector.tensor_scalar(a[:], pos_rows[c][:], MAGIC, None, OP.add)
                rr = rowp.tile([1, NT], F32, tag="rows", name=f"rr{c}")
                nc.vector.tensor_scalar(rr[:], a[:], MAGIC, None, OP.subtract)
                nc.sync.dma_start(pos_d[c:c + 1, b0:b0 + NT], rr[:])
            az = rowp.tile([1, NT], F32, tag="rows", name="az")
            nc.vector.tensor_scalar(az[:], pz[:], MAGIC, None, OP.add)
            rz = rowp.tile([1, NT], F32, tag="rows", name="rz")
            nc.vector.tensor_scalar(rz[:], az[:], MAGIC, None, OP.subtract)
            le63 = rowp.tile([1, NT], F32, tag="rows", name="le63")
            nc.vector.tensor_scalar(le63[:], pz[:], 63.0, None, OP.is_le)
            reach = rowp.tile([1, NT], F32, tag="rows", name="reach")
            nc.vector.tensor_tensor(reach[:], gez[:], le63[:], OP.mult)
            dd = rowp.tile([1, NT], F32, tag="rows", name="dd")
            nc.vector.tensor_scalar(dd[:], rz[:], 32.0, None, OP.subtract)
            ee = rowp.tile([1, NT], F32, tag="rows", name="ee")
            nc.vector.tensor_tensor(ee[:], reach[:], dd[:], OP.mult)
            npz = rowp.tile([1, NT], F32, tag="rows", name="npz")
            nc.vector.tensor_tensor(npz[:], rz[:], ee[:], OP.subtract)
            nc.sync.dma_start(pos_d[2:3, b0:b0 + NT], npz[:])

            # spawn counts from logit-space thresholds
            c1 = rowp.tile([1, NT], F32, tag="rows", name="c1")
            nc.vector.tensor_scalar(c1[:], g_sb[:], T0, None, OP.is_gt)
            c2 = rowp.tile([1, NT], F32, tag="rows", name="c2")
            nc.vector.tensor_scalar(c2[:], g_sb[:], T1, None, OP.is_gt)
            c3 = rowp.tile([1, NT], F32, tag="rows", name="c3")
            nc.vector.tensor_scalar(c3[:], g_sb[:], T2, None, OP.is_gt)
            s23 = rowp.tile([1, NT], F32, tag="rows", name="s23")
            nc.vector.tensor_tensor(s23[:], c2[:], c3[:], OP.add)
            s231 = rowp.tile([1, NT], F32, tag="rows", name="s231")
            nc.vector.tensor_scalar(s231[:], s23[:], 1.0, None, OP.add)
            cnt = rowp.tile([1, NT], F32, tag="rows", name="cnt")
            nc.vector.tensor_tensor(cnt[:], c1[:], s231[:], OP.mult)
            nc.sync.dma_start(ct_d[0:1, b0:b0 + NT], cnt[:])

            # spawn_frac = decided * se / (counts+1)
            cp1 = rowp.tile([1, NT], F32, tag="rows", name="cp1")
            nc.vector.tensor_scalar(cp1[:], cnt[:], 1.0, None, OP.add)
            # DVE has no divide op; reciprocal is exact for divisors 1/2/4 and
            # correctly rounded for 3 (<=1ulp off true division on frac scale)
            rec = rowp.tile([1, NT], F32, tag="rows", name="rec")
            nc.vector.reciprocal(rec[:], cp1[:])
            q = rowp.tile([1, NT], F32, tag="rows", name="q")
            nc.vector.tensor_tensor(q[:], se_sb[:], rec[:], OP.mult)
            frac = rowp.tile([1, NT], F32, tag="rows", name="frac")
            nc.vector.tensor_tensor(frac[:], c1[:], q[:], OP.mult)
            nc.sync.dma_start(fr_d[0:1, b0:b0 + NT], frac[:])

    nc.compile()
    return nc


def _get_prog():
    global _PROG
    if _PROG is None:
        _PROG = _build()
    return _PROG


def kernel(neuron_output, embedding_part, hidden_state, flow_age,
           w_ih0, w_hh0, b_ih0, b_hh0, w_ih1, w_hh1, b_ih1, b_hh1,
           ep_w1, ep_b1, ep_w2, ep_b2, pp_w1, pp_b1, pp_w2, pp_b2,
           sg_w1, sg_b1, sg_w2, sg_b2, se_w, se_b):
    global LAST_RES
    nc = _get_prog()

    f32 = np.float32
    x_full = np.concatenate([np.asarray(neuron_output, f32),
                             np.asarray(embedding_part, f32)], axis=1)
    hs = np.asarray(hidden_state, f32)
    fa_full = np.asarray(flow_age, f32)

    wih0t = np.ascontiguousarray(np.asarray(w_ih0, f32).T)
    whh0p = _pack_big(w_hh0)
    wih1p = _pack_big(w_ih1)
    whh1p = _pack_big(w_hh1)
    epw1p = _pack_big(ep_w1)
    ppw1t = _pack_kx(pp_w1)
    sgw1t = _pack_kx(sg_w1)
    sewt = _pack_kx(se_w)

    shared = {
        "ident": np.eye(128, dtype=f32),
        "wih0h": _hi(wih0t), "wih0l": _lo(wih0t),
        "whh0h": _hi(whh0p), "whh0l": _lo(whh0p),
        "wih1h": _hi(wih1p), "wih1l": _lo(wih1p),
        "whh1h": _hi(whh1p), "whh1l": _lo(whh1p),
        "epw1h": _hi(epw1p), "epw1l": _lo(epw1p),
        "epw2t": _pack_kx(ep_w2),
        "ppw1h": _hi(ppw1t), "ppw1l": _lo(ppw1t),
        "ppw2t": np.ascontiguousarray(np.asarray(pp_w2, f32).T),
        "sgw1h": _hi(sgw1t), "sgw1l": _lo(sgw1t),
        "sgw2t": np.ascontiguousarray(np.asarray(sg_w2, f32).T),
        "sewh": _hi(sewt), "sewl": _lo(sewt),
        "brz0": np.ascontiguousarray(
            (np.asarray(b_ih0, f32) + np.asarray(b_hh0, f32))[:2 * H]
            .reshape(16, 128).T),
        "bin0": np.ascontiguousarray(np.asarray(b_ih0, f32)[2 * H:].reshape(8, 128).T),
        "bhn0": np.ascontiguousarray(np.asarray(b_hh0, f32)[2 * H:].reshape(8, 128).T),
        "brz1": np.ascontiguousarray(
            (np.asarray(b_ih1, f32) + np.asarray(b_hh1, f32))[:2 * H]
            .reshape(16, 128).T),
        "bin1": np.ascontiguousarray(np.asarray(b_ih1, f32)[2 * H:].reshape(8, 128).T),
        "bhn1": np.ascontiguousarray(np.asarray(b_hh1, f32)[2 * H:].reshape(8, 128).T),
        "epb1": np.ascontiguousarray(np.asarray(ep_b1, f32).reshape(4, 128).T),
        "ppb1": np.asarray(pp_b1, f32).reshape(64, 1),
        "sgb1": np.asarray(sg_b1, f32).reshape(64, 1),
        "epb2": np.asarray(ep_b2, f32).reshape(1, 1),
        "ppb2": np.asarray(pp_b2, f32).reshape(1, 3),
        "sgb2": np.asarray(sg_b2, f32).reshape(1, 1),
        "seb": np.asarray(se_b, f32).reshape(1, 1),
    }

    in_maps = []
    for c in range(CORES):
        sl = slice(c * BS, (c + 1) * BS)
        m = dict(shared)
        m["x"] = np.ascontiguousarray(x_full[sl])
        for li, key in ((0, "h0"), (1, "h1")):
            hsl = np.ascontiguousarray(hs[li, sl])
            hi16 = hsl.astype(np.float16)
            m[key + "hi"] = hi16
            m[key + "lo"] = (hsl - hi16.astype(np.float32)).astype(np.float16)
        m["fa"] = np.ascontiguousarray(fa_full[sl].reshape(1, BS))
        in_maps.append(m)

    trace = bool(os.environ.get("EC_TRACE"))
    res = run_bass_kernel_spmd(nc, in_maps, list(range(CORES)), trace=trace)
    LAST_RES = res

    energy = np.concatenate([res.results[c]["energy"].reshape(BS, 1)
                             for c in range(CORES)], axis=0)
    pos = np.concatenate([np.ascontiguousarray(res.results[c]["pos"].T)
                          for c in range(CORES)], axis=0)
    frac = np.concatenate([res.results[c]["frac"].reshape(BS)
                           for c in range(CORES)], axis=0)
    counts = np.concatenate([res.results[c]["counts"].reshape(BS)
                             for c in range(CORES)], axis=0).astype(np.int32)
    term = (np.concatenate([res.results[c]["term"].reshape(BS)
                            for c in range(CORES)], axis=0) > 0.5)
    nh = np.stack([
        np.concatenate([res.results[c]["nh0"] for c in range(CORES)], axis=0),
        np.concatenate([res.results[c]["nh1"] for c in range(CORES)], axis=0),
    ], axis=0)
    return energy, pos, frac, counts, term, nh


# revision 16
# speedup vs baseline: 1.0768x; 1.0006x over previous
"""EnergyCarrier (2-layer GRU cell + heads) Trainium2 kernel.

Full inputs in, full outputs out. Data-parallel over 8 NeuronCores:
batch dim B=32768 sharded into 8x4096 rows; GRU/head weights replicated.

On-chip layout is feature-major ([feature_chunk=128 partitions, batch free])
so the GRU matmuls contract over partitions. Hidden-state inputs are split
into fp16 hi/lo on the host and loaded feature-major via hardware DMA
transpose (2-byte dtype, on the Sync HWDGE queue); outputs are transposed
back to batch-major with PE-mode transposes and DMA'd per 128x128 piece.

Precision scheme: the outputs include discrete quantities (is_terminated
bool, spawn_counts int, rounded positions) whose boundary flips dominate
the error budget, so bf16/f32r matmuls (~2^-12 operand truncation) are out.
Instead every large matmul runs as a 3-pass fp16 hi/lo split
(hi.hi + hi.lo + lo.hi accumulated in one PSUM bank): fp16 products are
exact in the PE's e10m23 accumulator and the dropped lo.lo term is
~2^-22 relative, giving fp32-grade results (measured ~8e-7 rel on HW) at
3 cycles/row instead of fp32's 4. Tiny M<=3 matmuls stay plain fp32.

Spawn decisions are computed from the spawn-gate pre-activation (thresholds
0, ln2, ln5 in logit space) instead of the sigmoid output: the ACT sigmoid
table has a 40-ULP budget, while the matmul pre-activation is fp32-grade.
Rounding uses the +/-1.5*2^23 magic-number trick (RNE, matches jnp.round's
half-to-even).
"""

import os
import sys
from contextlib import ExitStack

sys.path.insert(0, "/opt/trn_rl_repo")

import numpy as np

import concourse.bass as bass
import concourse.tile as tile
from concourse import bacc, mybir
from concourse.bass_utils import run_bass_kernel_spmd

AF = mybir.ActivationFunctionType
OP = mybir.AluOpType
F32 = mybir.dt.float32
F16 = mybir.dt.float16

B, H, IN = 32768, 1024, 33
CORES = 8
BS = B // CORES          # rows per core
NT = 512                 # batch-tile columns
NBT = BS // NT
KC = H // 128            # feature chunks
MAGIC = 12582912.0       # 1.5 * 2**23 -> RNE round-to-integer via add/sub
# spawn thresholds in pre-activation (logit) space:
# decided: sigmoid32(g) > 0.5 <=> g > 2^-23 (fp32 rounding boundary near 0.5)
T0 = float(np.float32(2.0 ** -23))
T1 = float(np.float32(np.log(2.0)))   # p > 2/3
T2 = float(np.float32(np.log(5.0)))   # p > 5/6

LAST_RES = None  # BassKernelResults of the most recent run (for test harness)
_PROG = None


def _pack_big(w):
    """[O,K] (O,K mult of 128) -> [O/128, 128, K]; [m,p,k*128+j] = w[m*128+j, k*128+p]."""
    o, k = w.shape
    m, kc = o // 128, k // 128
    w4 = np.asarray(w, np.float32).reshape(m, 128, kc, 128)
    return np.ascontiguousarray(w4.transpose(0, 3, 2, 1).reshape(m, 128, k))


def _pack_kx(w):
    """[O<=128, K] -> [128, (K/128)*O]; [p, k*O+j] = w[j, k*128+p]."""
    o, k = w.shape
    kc = k // 128
    w3 = np.asarray(w, np.float32).reshape(o, kc, 128)
    return np.ascontiguousarray(w3.transpose(2, 1, 0).reshape(128, kc * o))


def _hi(a):
    return np.asarray(a, np.float32).astype(np.float16)


def _lo(a):
    a = np.asarray(a, np.float32)
    return (a - a.astype(np.float16).astype(np.float32)).astype(np.float16)


def _build():
    nc = bacc.Bacc("TRN2", target_bir_lowering=False, debug=False,
                   num_devices=CORES)

    def din(name, shape, dt=F32):
        return nc.dram_tensor(name, list(shape), dt, kind="ExternalInput").ap()

    def dout(name, shape):
        return nc.dram_tensor(name, list(shape), F32, kind="ExternalOutput").ap()

    x_d = din("x", (BS, IN))
    h0hi_d = din("h0hi", (BS, H), F16)
    h0lo_d = din("h0lo", (BS, H), F16)
    h1hi_d = din("h1hi", (BS, H), F16)
    h1lo_d = din("h1lo", (BS, H), F16)
    fa_d = din("fa", (1, BS))
    ident_d = din("ident", (128, 128))
    wih0h_d = din("wih0h", (IN, 3 * H), F16)
    wih0l_d = din("wih0l", (IN, 3 * H), F16)
    whh0h_d = din("whh0h", (24, 128, H), F16)
    whh0l_d = din("whh0l", (24, 128, H), F16)
    wih1h_d = din("wih1h", (24, 128, H), F16)
    wih1l_d = din("wih1l", (24, 128, H), F16)
    whh1h_d = din("whh1h", (24, 128, H), F16)
    whh1l_d = din("whh1l", (24, 128, H), F16)
    epw1h_d = din("epw1h", (4, 128, H), F16)
    epw1l_d = din("epw1l", (4, 128, H), F16)
    epw2_d = din("epw2t", (128, 4))
    ppw1h_d = din("ppw1h", (128, KC * 64), F16)
    ppw1l_d = din("ppw1l", (128, KC * 64), F16)
    ppw2_d = din("ppw2t", (64, 3))
    sgw1h_d = din("sgw1h", (128, KC * 64), F16)
    sgw1l_d = din("sgw1l", (128, KC * 64), F16)
    sgw2_d = din("sgw2t", (64, 1))
    sewh_d = din("sewh", (128, KC), F16)
    sewl_d = din("sewl", (128, KC), F16)
    brz0_d = din("brz0", (128, 16))
    bin0_d = din("bin0", (128, 8))
    bhn0_d = din("bhn0", (128, 8))
    brz1_d = din("brz1", (128, 16))
    bin1_d = din("bin1", (128, 8))
    bhn1_d = din("bhn1", (128, 8))
    epb1_d = din("epb1", (128, 4))
    ppb1_d = din("ppb1", (64, 1))
    sgb1_d = din("sgb1", (64, 1))
    epb2_d = din("epb2", (1, 1))
    ppb2_d = din("ppb2", (1, 3))
    sgb2_d = din("sgb2", (1, 1))
    seb_d = din("seb", (1, 1))

    en_d = dout("energy", (1, BS))
    pos_d = dout("pos", (3, BS))
    fr_d = dout("frac", (1, BS))
    ct_d = dout("counts", (1, BS))
    tm_d = dout("term", (1, BS))
    nh0_d = dout("nh0", (BS, H))
    nh1_d = dout("nh1", (BS, H))

    with tile.TileContext(nc) as tc, ExitStack() as ctx:
        const = ctx.enter_context(tc.tile_pool(name="const", bufs=1))
        hload = ctx.enter_context(tc.tile_pool(name="hload", bufs=3))
        wmm = ctx.enter_context(tc.tile_pool(name="wmm", bufs=14))
        actp = ctx.enter_context(tc.tile_pool(name="actp", bufs=1))
        ewp = ctx.enter_context(tc.tile_pool(name="ewp", bufs=9))
        hnewp = ctx.enter_context(tc.tile_pool(name="hnewp", bufs=2))
        rowp = ctx.enter_context(tc.tile_pool(name="rowp", bufs=10))
        h64p = ctx.enter_context(tc.tile_pool(name="h64p", bufs=4))
        otp = ctx.enter_context(tc.tile_pool(name="otp", bufs=6))
        xtp = ctx.enter_context(tc.tile_pool(name="xtp", bufs=2))
        ep1p = ctx.enter_context(tc.tile_pool(name="ep1p", bufs=1))
        psmm = ctx.enter_context(tc.tile_pool(name="psmm", bufs=6, space="PSUM"))
        pstr = ctx.enter_context(tc.tile_pool(name="pstr", bufs=2, space="PSUM"))

        def ctile(ap_dram, shape, name, dt=F32):
            t = const.tile(shape, dt, tag=name, name=name)
            nc.sync.dma_start(t[:], ap_dram[:])
            return t

        ident_sb = ctile(ident_d, [128, 128], "ident_sb")
        wih0h_sb = ctile(wih0h_d, [IN, 3 * H], "wih0h_sb", F16)
        wih0l_sb = ctile(wih0l_d, [IN, 3 * H], "wih0l_sb", F16)
        ppw1h_sb = ctile(ppw1h_d, [128, KC * 64], "ppw1h_sb", F16)
        ppw1l_sb = ctile(ppw1l_d, [128, KC * 64], "ppw1l_sb", F16)
        sgw1h_sb = ctile(sgw1h_d, [128, KC * 64], "sgw1h_sb", F16)
        sgw1l_sb = ctile(sgw1l_d, [128, KC * 64], "sgw1l_sb", F16)
        sewh_sb = ctile(sewh_d, [128, KC], "sewh_sb", F16)
        sewl_sb = ctile(sewl_d, [128, KC], "sewl_sb", F16)
        epw2_sb = ctile(epw2_d, [128, 4], "epw2_sb")
        ppw2_sb = ctile(ppw2_d, [64, 3], "ppw2_sb")
        sgw2_sb = ctile(sgw2_d, [64, 1], "sgw2_sb")
        brz0_sb = ctile(brz0_d, [128, 16], "brz0_sb")
        bin0_sb = ctile(bin0_d, [128, 8], "bin0_sb")
        bhn0_sb = ctile(bhn0_d, [128, 8], "bhn0_sb")
        brz1_sb = ctile(brz1_d, [128, 16], "brz1_sb")
        bin1_sb = ctile(bin1_d, [128, 8], "bin1_sb")
        bhn1_sb = ctile(bhn1_d, [128, 8], "bhn1_sb")
        epb1_sb = ctile(epb1_d, [128, 4], "epb1_sb")
        ppb1_sb = ctile(ppb1_d, [64, 1], "ppb1_sb")
        sgb1_sb = ctile(sgb1_d, [64, 1], "sgb1_sb")
        epb2_sb = ctile(epb2_d, [1, 1], "epb2_sb")
        ppb2_sb = ctile(ppb2_d, [1, 3], "ppb2_sb")
        sgb2_sb = ctile(sgb2_d, [1, 1], "sgb2_sb")
        seb_sb = ctile(seb_d, [1, 1], "seb_sb")

        copy_ctr = [0]

        def pcopy(dst, src):
            # alternate DVE/ACT so neither engine owns all psum drains
            if copy_ctr[0] % 2 == 0:
                nc.vector.tensor_copy(dst, src)
            else:
                nc.scalar.copy(dst, src)
            copy_ctr[0] += 1

        def split_from(dst_hi, dst_lo, src):
            # hi = fp16(src); lo = fp16(src - hi). src may be PSUM (1 psum
            # operand per DVE op). Exact to ~2^-22 regardless of rounding.
            pcopy(dst_hi, src)
            nc.vector.scalar_tensor_tensor(dst_lo, dst_hi, -1.0, src,
                                           OP.mult, OP.add)

        def load_transposed(dst_hi, dst_lo, hi_dram, lo_dram, b0):
            # dst[p, k*NT + j] <- src[b0+j, 128k+p]; host pre-split fp16 hi/lo,
            # loaded feature-major via hardware DMA transpose (2-byte dtype)
            for k in range(KC):
                nc.sync.dma_start_transpose(
                    dst_hi[:, k * NT:(k + 1) * NT],
                    hi_dram[b0:b0 + NT, k * 128:(k + 1) * 128])
                nc.sync.dma_start_transpose(
                    dst_lo[:, k * NT:(k + 1) * NT],
                    lo_dram[b0:b0 + NT, k * 128:(k + 1) * 128])

        def mm3(ps, wh, wl, xh, xl, start, stop):
            nc.tensor.matmul(ps, wh, xh, start=start, stop=False)
            nc.tensor.matmul(ps, wh, xl, start=False, stop=False)
            nc.tensor.matmul(ps, wl, xh, start=False, stop=stop)

        for bt in range(NBT):
            b0 = bt * NT

            # ---- inputs: h_prev^T then x^T (fp16 hi/lo pairs) ----
            h0Th = actp.tile([128, KC * NT], F16, tag="h0Th", name="h0Th",
                             bufs=2)
            h0Tl = actp.tile([128, KC * NT], F16, tag="h0Tl", name="h0Tl",
                             bufs=2)
            load_transposed(h0Th, h0Tl, h0hi_d, h0lo_d, b0)
            h1Th = actp.tile([128, KC * NT], F16, tag="h1Th", name="h1Th",
                             bufs=2)
            h1Tl = actp.tile([128, KC * NT], F16, tag="h1Tl", name="h1Tl",
                             bufs=2)
            load_transposed(h1Th, h1Tl, h1hi_d, h1lo_d, b0)

            xTh = xtp.tile([IN, NT], F16, tag="xTh", name="xTh")
            xTl = xtp.tile([IN, NT], F16, tag="xTl", name="xTl")
            for i in range(NT // 128):
                xb = hload.tile([128, IN], F32, tag="xb", name="xb")
                nc.sync.dma_start(xb[:], x_d[b0 + i * 128:b0 + (i + 1) * 128, :])
                psx = pstr.tile([128, 128], F32, tag="tr", name="ps_tr")
                nc.tensor.transpose(psx[0:IN, :], xb[:], ident_sb[:])
                sl = slice(i * 128, (i + 1) * 128)
                split_from(xTh[:, sl], xTl[:, sl], psx[0:IN, :])

            h0nTh = actp.tile([128, KC * NT], F16, tag="h0nTh", name="h0nTh")
            h0nTl = actp.tile([128, KC * NT], F16, tag="h0nTl", name="h0nTl")
            h1nTh = actp.tile([128, KC * NT], F16, tag="h1nTh", name="h1nTh")
            h1nTl = actp.tile([128, KC * NT], F16, tag="h1nTl", name="h1nTl")

            def gru_layer(layer, f, hTh, hTl, hnTh, hnTl, nh_dram):
                m_r, m_z, m_n = f, 8 + f, 16 + f
                if layer == 0:
                    wih_h, wih_l = whh0h_d, whh0l_d
                    brz, binb, bhnb = brz0_sb, bin0_sb, bhn0_sb
                else:
                    wih_h, wih_l = whh1h_d, whh1l_d
                    brz, binb, bhnb = brz1_sb, bin1_sb, bhn1_sb

                wrh = wmm.tile([128, H], F16, tag="w", name="wrh")
                nc.sync.dma_start(wrh[:], wih_h[m_r])
                wrl = wmm.tile([128, H], F16, tag="w", name="wrl")
                nc.sync.dma_start(wrl[:], wih_l[m_r])
                wzh = wmm.tile([128, H], F16, tag="w", name="wzh")
                nc.sync.dma_start(wzh[:], wih_h[m_z])
                wzl = wmm.tile([128, H], F16, tag="w", name="wzl")
                nc.sync.dma_start(wzl[:], wih_l[m_z])
                wnh = wmm.tile([128, H], F16, tag="w", name="wnh")
                nc.sync.dma_start(wnh[:], wih_h[m_n])
                wnl = wmm.tile([128, H], F16, tag="w", name="wnl")
                nc.sync.dma_start(wnl[:], wih_l[m_n])
                if layer == 1:
                    virh = wmm.tile([128, H], F16, tag="w", name="virh")
                    nc.sync.dma_start(virh[:], wih1h_d[m_r])
                    virl = wmm.tile([128, H], F16, tag="w", name="virl")
                    nc.sync.dma_start(virl[:], wih1l_d[m_r])
                    vizh = wmm.tile([128, H], F16, tag="w", name="vizh")
                    nc.sync.dma_start(vizh[:], wih1h_d[m_z])
                    vizl = wmm.tile([128, H], F16, tag="w", name="vizl")
                    nc.sync.dma_start(vizl[:], wih1l_d[m_z])
                    vinh = wmm.tile([128, H], F16, tag="w", name="vinh")
                    nc.sync.dma_start(vinh[:], wih1h_d[m_n])
                    vinl = wmm.tile([128, H], F16, tag="w", name="vinl")
                    nc.sync.dma_start(vinl[:], wih1l_d[m_n])

                def xpart(ps, mm, start):
                    # layer 0: x contribution (K=33); layer 1: h0_new contribution
                    if layer == 0:
                        sl = slice(mm * 128, (mm + 1) * 128)
                        mm3(ps, wih0h_sb[:, sl], wih0l_sb[:, sl], xTh[:], xTl[:],
                            start, False)
                    else:
                        w_h, w_l = {m_r: (virh, virl), m_z: (vizh, vizl),
                                    m_n: (vinh, vinl)}[mm]
                        for k in range(KC):
                            mm3(ps[:], w_h[:, k * 128:(k + 1) * 128],
                                w_l[:, k * 128:(k + 1) * 128],
                                h0nTh[:, k * NT:(k + 1) * NT],
                                h0nTl[:, k * NT:(k + 1) * NT],
                                start and k == 0, False)

                def hpart(ps, wh, wl, stop):
                    for k in range(KC):
                        mm3(ps[:], wh[:, k * 128:(k + 1) * 128],
                            wl[:, k * 128:(k + 1) * 128],
                            hTh[:, k * NT:(k + 1) * NT],
                            hTl[:, k * NT:(k + 1) * NT],
                            False, stop and k == KC - 1)

                ps_r = psmm.tile([128, NT], F32, tag="mm", name="ps_r")
                xpart(ps_r, m_r, True)
                hpart(ps_r, wrh, wrl, True)
                r_sb = ewp.tile([128, NT], F32, tag="ew", name="r_sb")
                nc.scalar.activation(r_sb[:], ps_r[:], AF.Sigmoid,
                                     bias=brz[:, f:f + 1])

                ps_z = psmm.tile([128, NT], F32, tag="mm", name="ps_z")
                xpart(ps_z, m_z, True)
                hpart(ps_z, wzh, wzl, True)
                z_sb = ewp.tile([128, NT], F32, tag="ew", name="z_sb")
                nc.scalar.activation(z_sb[:], ps_z[:], AF.Sigmoid,
                                     bias=brz[:, 8 + f:9 + f])

                ps_in = psmm.tile([128, NT], F32, tag="mm", name="ps_in")
                if layer == 0:
                    sl = slice(m_n * 128, (m_n + 1) * 128)
                    mm3(ps_in[:], wih0h_sb[:, sl], wih0l_sb[:, sl], xTh[:], xTl[:],
                        True, True)
                else:
                    for k in range(KC):
                        mm3(ps_in[:], vinh[:, k * 128:(k + 1) * 128],
                            vinl[:, k * 128:(k + 1) * 128],
                            h0nTh[:, k * NT:(k + 1) * NT],
                            h0nTl[:, k * NT:(k + 1) * NT],
                            k == 0, k == KC - 1)
                ps_hn = psmm.tile([128, NT], F32, tag="mm", name="ps_hn")
                for k in range(KC):
                    mm3(ps_hn[:], wnh[:, k * 128:(k + 1) * 128],
                        wnl[:, k * 128:(k + 1) * 128],
                        hTh[:, k * NT:(k + 1) * NT],
                        hTl[:, k * NT:(k + 1) * NT],
                        k == 0, k == KC - 1)

                t1 = ewp.tile([128, NT], F32, tag="ew", name="t1")
                nc.vector.scalar_tensor_tensor(t1[:], ps_hn[:], bhnb[:, f:f + 1],
                                               r_sb[:], OP.add, OP.mult)
                t2 = ewp.tile([128, NT], F32, tag="ew", name="t2")
                nc.vector.tensor_tensor(t2[:], t1[:], ps_in[:], OP.add)
                n_sb = ewp.tile([128, NT], F32, tag="ew", name="n_sb")
                nc.scalar.activation(n_sb[:], t2[:], AF.Tanh, bias=binb[:, f:f + 1])

                fsl = slice(f * NT, (f + 1) * NT)
                u_sb = ewp.tile([128, NT], F32, tag="ew", name="u_sb")
                nc.vector.tensor_tensor(u_sb[:], hTh[:, fsl], hTl[:, fsl], OP.add)
                d_sb = ewp.tile([128, NT], F32, tag="ew", name="d_sb")
                nc.vector.tensor_tensor(d_sb[:], u_sb[:], n_sb[:], OP.subtract)
                e_sb = ewp.tile([128, NT], F32, tag="ew", name="e_sb")
                nc.vector.tensor_tensor(e_sb[:], z_sb[:], d_sb[:], OP.mult)
                hnew = hnewp.tile([128, NT], F32, tag="hnew", name="hnew")
                nc.vector.tensor_tensor(hnew[:], n_sb[:], e_sb[:], OP.add)

                split_from(hnTh[:, fsl], hnTl[:, fsl], hnew[:])

                # transpose h_new back to batch-major; DMA each 128x128 piece
                # straight to DRAM (512B bursts, no staging buffer)
                for i in range(NT // 128):
                    ps = pstr.tile([128, 128], F32, tag="tr", name="ps_tr")
                    nc.tensor.transpose(ps[:], hnew[:, i * 128:(i + 1) * 128],
                                        ident_sb[:])
                    ot = otp.tile([128, 128], F32, tag="ot", name="ot")
                    pcopy(ot[:], ps[:])
                    nc.sync.dma_start(
                        nh_dram[b0 + i * 128:b0 + (i + 1) * 128,
                                f * 128:(f + 1) * 128], ot[:])

            # ---- layer 0 ----
            for f in range(KC):
                gru_layer(0, f, h0Th, h0Tl, h0nTh, h0nTl, nh0_d)

            # ---- layer 1 ----
            for f in range(KC):
                gru_layer(1, f, h1Th, h1Tl, h1nTh, h1nTl, nh1_d)

            # ---- heads (all gelus first: one ACT table-set switch each way) ----
            ep1T = ep1p.tile([128, 4 * NT], F32, tag="ep1T", name="ep1T")
            for mo in range(4):
                weh = wmm.tile([128, H], F16, tag="w", name="weh")
                nc.sync.dma_start(weh[:], epw1h_d[mo])
                wel = wmm.tile([128, H], F16, tag="w", name="wel")
                nc.sync.dma_start(wel[:], epw1l_d[mo])
                ps_e = psmm.tile([128, NT], F32, tag="mm", name="ps_e")
                for k in range(KC):
                    mm3(ps_e[:], weh[:, k * 128:(k + 1) * 128],
                        wel[:, k * 128:(k + 1) * 128],
                        h1nTh[:, k * NT:(k + 1) * NT],
                        h1nTl[:, k * NT:(k + 1) * NT],
                        k == 0, k == KC - 1)
                nc.scalar.activation(ep1T[:, mo * NT:(mo + 1) * NT], ps_e[:],
                                     AF.Gelu, bias=epb1_sb[:, mo:mo + 1])

            ps_p = psmm.tile([64, NT], F32, tag="mm", name="ps_p")
            for k in range(KC):
                mm3(ps_p[:], ppw1h_sb[:, k * 64:(k + 1) * 64],
                    ppw1l_sb[:, k * 64:(k + 1) * 64],
                    h1nTh[:, k * NT:(k + 1) * NT],
                    h1nTl[:, k * NT:(k + 1) * NT],
                    k == 0, k == KC - 1)
            pp1_sb = h64p.tile([64, NT], F32, tag="h64", name="pp1_sb")
            nc.scalar.activation(pp1_sb[:], ps_p[:], AF.Gelu, bias=ppb1_sb[:])

            ps_g = psmm.tile([64, NT], F32, tag="mm", name="ps_g")
            for k in range(KC):
                mm3(ps_g[:], sgw1h_sb[:, k * 64:(k + 1) * 64],
                    sgw1l_sb[:, k * 64:(k + 1) * 64],
                    h1nTh[:, k * NT:(k + 1) * NT],
                    h1nTl[:, k * NT:(k + 1) * NT],
                    k == 0, k == KC - 1)
            sg1_sb = h64p.tile([64, NT], F32, tag="h64", name="sg1_sb")
            nc.scalar.activation(sg1_sb[:], ps_g[:], AF.Gelu, bias=sgb1_sb[:])

            # energy head tail (fp32, M=1)
            ps_ev = psmm.tile([1, NT], F32, tag="mm", name="ps_ev")
            for mo in range(4):
                nc.tensor.matmul(ps_ev[:], epw2_sb[:, mo:mo + 1],
                                 ep1T[:, mo * NT:(mo + 1) * NT],
                                 start=(mo == 0), stop=(mo == 3))
            en_sb = rowp.tile([1, NT], F32, tag="rows", name="en_sb")
            nc.scalar.activation(en_sb[:], ps_ev[:], AF.Tanh, bias=epb2_sb[:])
            nc.sync.dma_start(en_d[0:1, b0:b0 + NT], en_sb[:])

            # position head tail: one M=1 matmul per coordinate (keeps every
            # row-op operand at partition base 0)
            pos_rows = []
            for c in range(3):
                ps_pc = psmm.tile([1, NT], F32, tag="mm", name="ps_pc")
                nc.tensor.matmul(ps_pc[:], ppw2_sb[:, c:c + 1], pp1_sb[:],
                                 start=True, stop=True)
                pn = rowp.tile([1, NT], F32, tag="rows", name=f"pn{c}")
                nc.scalar.activation(pn[:], ps_pc[:], AF.Tanh,
                                     bias=ppb2_sb[:, c:c + 1])
                # pred = ((pos+1)*0.5)*63  (first two ops exact, one rounding)
                s12 = rowp.tile([1, NT], F32, tag="rows", name=f"s12_{c}")
                nc.vector.tensor_scalar(s12[:], pn[:], 1.0, 0.5, OP.add, OP.mult)
                s3 = rowp.tile([1, NT], F32, tag="rows", name=f"s3_{c}")
                nc.vector.tensor_scalar(s3[:], s12[:], 63.0, None, OP.mult)
                pos_rows.append(s3)

            # spawn energy tail (fp16 3-pass, M=1)
            ps_se = psmm.tile([1, NT], F32, tag="mm", name="ps_se")
            for k in range(KC):
                mm3(ps_se[:], sewh_sb[:, k:k + 1], sewl_sb[:, k:k + 1],
                    h1nTh[:, k * NT:(k + 1) * NT],
                    h1nTl[:, k * NT:(k + 1) * NT],
                    k == 0, k == KC - 1)
            se_sb = rowp.tile([1, NT], F32, tag="rows", name="se_sb")
            nc.scalar.activation(se_sb[:], ps_se[:], AF.Tanh, bias=seb_sb[:])

            # spawn gate pre-activation (fp32, M=1)
            ps_g2 = psmm.tile([1, NT], F32, tag="mm", name="ps_g2")
            nc.tensor.matmul(ps_g2[:], sgw2_sb[:], sg1_sb[:], start=True, stop=True)
            g_sb = rowp.tile([1, NT], F32, tag="rows", name="g_sb")
            nc.vector.tensor_scalar(g_sb[:], ps_g2[:], sgb2_sb[:], None, OP.add)

            # flow-age z bias: 0.5 + fa*0.1
            fa_row = rowp.tile([1, NT], F32, tag="rows", name="fa_row")
            nc.sync.dma_start(fa_row[:], fa_d[0:1, b0:b0 + NT])
            faz = rowp.tile([1, NT], F32, tag="rows", name="faz")
            nc.vector.tensor_scalar(faz[:], fa_row[:], 0.1, 0.5, OP.mult, OP.add)
            pz = rowp.tile([1, NT], F32, tag="rows", name="pz")
            nc.vector.tensor_tensor(pz[:], pos_rows[2][:], faz[:], OP.add)

            # termination: oob_xy | (pz >= 32)   [reached|oob_z == pz>=32]
            cmp_or = None
            for c in range(2):
                lt = rowp.tile([1, NT], F32, tag="rows", name=f"lt{c}")
                nc.vector.tensor_scalar(lt[:], pos_rows[c][:], 0.0, None, OP.is_lt)
                ge = rowp.tile([1, NT], F32, tag="rows", name=f"ge{c}")
                nc.vector.tensor_scalar(ge[:], pos_rows[c][:], 64.0, None, OP.is_ge)
                oo = rowp.tile([1, NT], F32, tag="rows", name=f"oo{c}")
                nc.vector.tensor_tensor(oo[:], lt[:], ge[:], OP.logical_or)
                if cmp_or is None:
                    cmp_or = oo
                else:
                    oo2 = rowp.tile([1, NT], F32, tag="rows", name="oo2")
                    nc.vector.tensor_tensor(oo2[:], cmp_or[:], oo[:], OP.logical_or)
                    cmp_or = oo2
            gez = rowp.tile([1, NT], F32, tag="rows", name="gez")
            nc.vector.tensor_scalar(gez[:], pz[:], 32.0, None, OP.is_ge)
            term = rowp.tile([1, NT], F32, tag="rows", name="term")
            nc.vector.tensor_tensor(term[:], cmp_or[:], gez[:], OP.logical_or)
            nc.sync.dma_start(tm_d[0:1, b0:b0 + NT], term[:])

            # next_position: RNE round; z overridden to 32 where reached
            for c in range(2):
                a = rowp.tile([1, NT], F32, tag="rows", name=f"a{c}")
                nc.vector.tensor_scalar(a[:], pos_rows[c][:], MAGIC, None, OP.add)
                rr = rowp.tile([1, NT], F32, tag="rows", name=f"rr{c}")
                nc.vector.tensor_scalar(rr[:], a[:], MAGIC, None, OP.subtract)
                nc.sync.dma_start(pos_d[c:c + 1, b0:b0 + NT], rr[:])
            az = rowp.tile([1, NT], F32, tag="rows", name="az")
            nc.vector.tensor_scalar(az[:], pz[:], MAGIC, None, OP.add)
            rz = rowp.tile([1, NT], F32, tag="rows", name="rz")
            nc.vector.tensor_scalar(rz[:], az[:], MAGIC, None, OP.subtract)
            le63 = rowp.tile([1, NT], F32, tag="rows", name="le63")
            nc.vector.tensor_scalar(le63[:], pz[:], 63.0, None, OP.is_le)
            reach = rowp.tile([1, NT], F32, tag="rows", name="reach")
            nc.vector.tensor_tensor(reach[:], gez[:], le63[:], OP.mult)
            dd = rowp.tile([1, NT], F32, tag="rows", name="dd")
            nc.vector.tensor_scalar(dd[:], rz[:], 32.0, None, OP.subtract)
            ee = rowp.tile([1, NT], F32, tag="rows", name="ee")
            nc.vector.tensor_tensor(ee[:], reach[:], dd[:], OP.mult)
            npz = rowp.tile([1, NT], F32, tag="rows", name="npz")
            nc.vector.tensor_tensor(npz[:], rz[:], ee[:], OP.subtract)
            nc.sync.dma_start(pos_d[2:3, b0:b0 + NT], npz[:])

            # spawn counts from logit-space thresholds
            c1 = rowp.tile([1, NT], F32, tag="rows", name="c1")
            nc.vector.tensor_scalar(c1[:], g_sb[:], T0, None, OP.is_gt)
            c2 = rowp.tile([1, NT], F32, tag="rows", name="c2")
            nc.vector.tensor_scalar(c2[:], g_sb[:], T1, None, OP.is_gt)
            c3 = rowp.tile([1, NT], F32, tag="rows", name="c3")
            nc.vector.tensor_scalar(c3[:], g_sb[:], T2, None, OP.is_gt)
            s23 = rowp.tile([1, NT], F32, tag="rows", name="s23")
            nc.vector.tensor_tensor(s23[:], c2[:], c3[:], OP.add)
            s231 = rowp.tile([1, NT], F32, tag="rows", name="s231")
            nc.vector.tensor_scalar(s231[:], s23[:], 1.0, None, OP.add)
            cnt = rowp.tile([1, NT], F32, tag="rows", name="cnt")
            nc.vector.tensor_tensor(cnt[:], c1[:], s231[:], OP.mult)
            nc.sync.dma_start(ct_d[0:1, b0:b0 + NT], cnt[:])

            # spawn_frac = decided * se / (counts+1)
            cp1 = rowp.tile([1, NT], F32, tag="rows", name="cp1")
            nc.vector.tensor_scalar(cp1[:], cnt[:], 1.0, None, OP.add)
            # DVE has no divide op; reciprocal is exact for divisors 1/2/4 and
            # correctly rounded for 3 (<=1ulp off true division on frac scale)
            rec = rowp.tile([1, NT], F32, tag="rows", name="rec")
            nc.vector.reciprocal(rec[:], cp1[:])
            q = rowp.tile([1, NT], F32, tag="rows", name="q")
            nc.vector.tensor_tensor(q[:], se_sb[:], rec[:], OP.mult)
            frac = rowp.tile([1, NT], F32, tag="rows", name="frac")
            nc.vector.tensor_tensor(frac[:], c1[:], q[:], OP.mult)
            nc.sync.dma_start(fr_d[0:1, b0:b0 + NT], frac[:])

    nc.compile()
    return nc


def _get_prog():
    global _PROG
    if _PROG is None:
        _PROG = _build()
    return _PROG


def kernel(neuron_output, embedding_part, hidden_state, flow_age,
           w_ih0, w_hh0, b_ih0, b_hh0, w_ih1, w_hh1, b_ih1, b_hh1,
           ep_w1, ep_b1, ep_w2, ep_b2, pp_w1, pp_b1, pp_w2, pp_b2,
           sg_w1, sg_b1, sg_w2, sg_b2, se_w, se_b):
    global LAST_RES
    nc = _get_prog()

    f32 = np.float32
    x_full = np.concatenate([np.asarray(neuron_output, f32),
                             np.asarray(embedding_part, f32)], axis=1)
    hs = np.asarray(hidden_state, f32)
    fa_full = np.asarray(flow_age, f32)

    wih0t = np.ascontiguousarray(np.asarray(w_ih0, f32).T)
    whh0p = _pack_big(w_hh0)
    wih1p = _pack_big(w_ih1)
    whh1p = _pack_big(w_hh1)
    epw1p = _pack_big(ep_w1)
    ppw1t = _pack_kx(pp_w1)
    sgw1t = _pack_kx(sg_w1)
    sewt = _pack_kx(se_w)

    shared = {
        "ident": np.eye(128, dtype=f32),
        "wih0h": _hi(wih0t), "wih0l": _lo(wih0t),
        "whh0h": _hi(whh0p), "whh0l": _lo(whh0p),
        "wih1h": _hi(wih1p), "wih1l": _lo(wih1p),
        "whh1h": _hi(whh1p), "whh1l": _lo(whh1p),
        "epw1h": _hi(epw1p), "epw1l": _lo(epw1p),
        "epw2t": _pack_kx(ep_w2),
        "ppw1h": _hi(ppw1t), "ppw1l": _lo(ppw1t),
        "ppw2t": np.ascontiguousarray(np.asarray(pp_w2, f32).T),
        "sgw1h": _hi(sgw1t), "sgw1l": _lo(sgw1t),
        "sgw2t": np.ascontiguousarray(np.asarray(sg_w2, f32).T),
        "sewh": _hi(sewt), "sewl": _lo(sewt),
        "brz0": np.ascontiguousarray(
            (np.asarray(b_ih0, f32) + np.asarray(b_hh0, f32))[:2 * H]
            .reshape(16, 128).T),
        "bin0": np.ascontiguousarray(np.asarray(b_ih0, f32)[2 * H:].reshape(8, 128).T),
        "bhn0": np.ascontiguousarray(np.asarray(b_hh0, f32)[2 * H:].reshape(8, 128).T),
        "brz1": np.ascontiguousarray(
            (np.asarray(b_ih1, f32) + np.asarray(b_hh1, f32))[:2 * H]
            .reshape(16, 128).T),
        "bin1": np.ascontiguousarray(np.asarray(b_ih1, f32)[2 * H:].reshape(8, 128).T),
        "bhn1": np.ascontiguousarray(np.asarray(b_hh1, f32)[2 * H:].reshape(8, 128).T),
        "epb1": np.ascontiguousarray(np.asarray(ep_b1, f32).reshape(4, 128).T),
        "ppb1": np.asarray(pp_b1, f32).reshape(64, 1),
        "sgb1": np.asarray(sg_b1, f32).reshape(64, 1),
        "epb2": np.asarray(ep_b2, f32).reshape(1, 1),
        "ppb2": np.asarray(pp_b2, f32).reshape(1, 3),
        "sgb2": np.asarray(sg_b2, f32).reshape(1, 1),
        "seb": np.asarray(se_b, f32).reshape(1, 1),
    }

    in_maps = []
    for c in range(CORES):
        sl = slice(c * BS, (c + 1) * BS)
        m = dict(shared)
        m["x"] = np.ascontiguousarray(x_full[sl])
        for li, key in ((0, "h0"), (1, "h1")):
            hsl = np.ascontiguousarray(hs[li, sl])
            hi16 = hsl.astype(np.float16)
            m[key + "hi"] = hi16
            m[key + "lo"] = (hsl - hi16.astype(np.float32)).astype(np.float16)
        m["fa"] = np.ascontiguousarray(fa_full[sl].reshape(1, BS))
        in_maps.append(m)

    trace = bool(os.environ.get("EC_TRACE"))
    res = run_bass_kernel_spmd(nc, in_maps, list(range(CORES)), trace=trace)
    LAST_RES = res

    energy = np.concatenate([res.results[c]["energy"].reshape(BS, 1)
                             for c in range(CORES)], axis=0)
    pos = np.concatenate([np.ascontiguousarray(res.results[c]["pos"].T)
                          for c in range(CORES)], axis=0)
    frac = np.concatenate([res.results[c]["frac"].reshape(BS)
                           for c in range(CORES)], axis=0)
    counts = np.concatenate([res.results[c]["counts"].reshape(BS)
                             for c in range(CORES)], axis=0).astype(np.int32)
    term = (np.concatenate([res.results[c]["term"].reshape(BS)
                            for c in range(CORES)], axis=0) > 0.5)
    nh = np.stack([
        np.concatenate([res.results[c]["nh0"] for c in range(CORES)], axis=0),
        np.concatenate([res.results[c]["nh1"] for c in range(CORES)], axis=0),
    ], axis=0)
    return energy, pos, frac, counts, term, nh
